# revision 7
# baseline (speedup 1.0000x reference)
"""CaptionDecoder Trainium2 kernel: 8-core SPMD.

Strategy:
  - Recurrence (attention + LSTM, T=32 steps) is batch-sharded: each core
    owns 4 of 32 batch rows; all weights replicated. No collectives.
  - Hidden states for all steps are then AllGathered (1MB, one collective)
    and the vocab projection [1024,512]@[512,32000] is vocab-sharded:
    each core computes a 4000-wide vocab slice for the full batch.
  - All matmul operands are bf16 (fp32 PSUM accumulation).
  - Per-step critical path minimized:
      * energy bias-add via tensor_scalar (per-partition scalar) instead
        of broadcast tensor_tensor
      * softmax uses real Exp (+accum_out running sum); LSTM sigmoids are
        computed as 0.5*tanh(x/2)+0.5 so every ACT call stays in the
        exp_and_others table set (tanh+exp) -> zero table reloads
      * state is kept pre-doubled (H=2h, C=2c) with compensating 0.5x in
        the h-consuming weights so the sigmoid affine folds into
        scalar_tensor_tensor ops
      * ctx^T computed directly via per-(b,eh) matmuls with features as
        the stationary operand (no [4,E] detour / masked copies)
      * gate matmuls accumulate h-part and ctx-part into one PSUM group
  - Output written bf16 (host upcasts): halves the 16MB/core output DMA.
"""
import sys
import numpy as np
import ml_dtypes

sys.path.insert(0, "/opt/trn_rl_repo")

from contextlib import ExitStack

import concourse.bass as bass
import concourse.tile as tile
from concourse import bacc, mybir
from concourse.bass_utils import run_bass_kernel_spmd

BF16 = mybir.dt.bfloat16
F32 = mybir.dt.float32
AF = mybir.ActivationFunctionType
ALU = mybir.AluOpType

E = 512
H = 512
V = 32000
B = 32
P = 196
T = 32
N_CORES = 8
BL = B // N_CORES          # 4 batch rows per core
VS = V // N_CORES          # 4000 vocab per core
KH = 4                     # 128-chunks of E / H
GH = 16                    # 128-chunks of 4H
PC0, PC1 = 128, P - 128    # pixel chunks 128 + 68

bf16 = ml_dtypes.bfloat16


def _to_tiles(mat_T):
    """[K, M] -> [128, K//128, M] (partition-major K tiles)."""
    Kdim, M = mat_T.shape
    return mat_T.reshape(Kdim // 128, 128, M).transpose(1, 0, 2)


def build_nc(n_cores):
    nc = bacc.Bacc(
        "TRN2",
        target_bir_lowering=False,
        debug=False,
        enable_asserts=False,
        num_devices=n_cores,
    )

    def inp(name, shape, dt=BF16):
        return nc.declare_dram_parameter(name, list(shape), dt, isOutput=False).ap()

    # Per-core sharded inputs
    featT_p = inp("featT", [128, KH, BL, P])            # features^T [el, eh, b, p]
    feat_p = inp("feat", [128, BL, 2, E])               # [p_lo, b, pc, e] (pc1 rows>=68 pad)
    embT_p = inp("embT", [128, KH, BL * T])             # gathered emb^T [el, eh, (b,t)]
    linWT_p = inp("linWT", [128, KH, VS])               # lin_W shard^T (x0.5)
    linb_p = inp("linb", [128, VS], F32)                # host-expanded over partitions
    # Replicated weights (gate rows permuted to [g,f,i,o]; h-consumers x0.5)
    WfT_p = inp("WfT", [128, KH, H])
    WhT_p = inp("WhT", [128, KH, H])
    WcombT_p = inp("WcombT", [128, 2 * KH, 4 * H])      # [ctx;h] -> gates
    WembT_p = inp("WembT", [128, KH, 4 * H])
    ihWT_p = inp("ihWT", [128, KH, H])                  # inith_W^T * 2/P
    icWT_p = inp("icWT", [128, KH, H])
    ihb_p = inp("ihb", [128, KH, BL], F32)              # x2
    icb_p = inp("icb", [128, KH, BL], F32)              # x2
    attnb_p = inp("attnb", [128, KH, BL], F32)
    bcomb_p = inp("bcomb", [128, GH, 1], F32)           # (b_ih+b_hh) permuted
    vmaskT_p = inp("vmaskT", [128, KH, BL, BL])         # v masked per batch col
    eye4_p = inp("eye4", [4, 4])

    out_p = nc.declare_dram_parameter(
        "out", [n_cores * BL * T, VS], BF16, isOutput=True
    ).ap()

    hbounce = nc.dram_tensor("hbounce", [128, KH * BL * T], BF16).ap()
    hgath = nc.dram_tensor(
        "hgath", [n_cores * 128, KH * BL * T], BF16, addr_space="Shared"
    ).ap()

    with tile.TileContext(nc) as tc, ExitStack() as ctx:
        const = ctx.enter_context(tc.tile_pool(name="const", bufs=1))
        state = ctx.enter_context(tc.tile_pool(name="state", bufs=1))
        work = ctx.enter_context(tc.tile_pool(name="work", bufs=2))

        # ---- persistent SBUF ----
        feat_sb = const.tile([128, BL, 2, E], BF16)
        WhT_sb = const.tile([128, KH, H], BF16)
        WcombT_sb = const.tile([128, 2 * KH, 4 * H], BF16)
        vmaskT_sb = const.tile([128, KH, BL, BL], BF16)
        eye4_sb = const.tile([4, 4], BF16)
        attnb_sb = const.tile([128, KH, BL], F32)
        featproT = const.tile([128, KH, BL, P], BF16)   # Wf@features^T
        embprojT = const.tile([128, GH, BL, T], F32)    # Wemb@emb^T + b (permuted)
        linWT_sb = const.tile([128, KH, VS], BF16)
        linb_sb = const.tile([128, VS], F32)

        ctxhT = state.tile([128, KH, BL], BF16)         # ctx^T gate input
        h0_sb = state.tile([128, KH, BL], BF16)         # H0 = 2h0
        cT = state.tile([128, KH, BL], F32)             # C = 2c
        h_histT = state.tile([128, KH, BL, T], BF16)    # H = 2h history

        nc.sync.dma_start(feat_sb[:], feat_p[:])
        nc.sync.dma_start(WhT_sb[:], WhT_p[:])
        nc.sync.dma_start(vmaskT_sb[:], vmaskT_p[:])
        nc.sync.dma_start(eye4_sb[:], eye4_p[:])
        nc.sync.dma_start(attnb_sb[:], attnb_p[:])
        nc.sync.dma_start(WcombT_sb[:], WcombT_p[:])

        # ---- precompute ----
        with (
            tc.tile_pool(name="pre", bufs=1) as pre,
            tc.tile_pool(name="prepsum", bufs=1, space="PSUM") as prepsum,
        ):
            featT_sb = pre.tile([128, KH, BL, P], BF16)
            embT_sb = pre.tile([128, KH, BL * T], BF16)
            WfT_sb = pre.tile([128, KH, H], BF16)
            WembT_sb = pre.tile([128, KH, 4 * H], BF16)
            ihWT_sb = pre.tile([128, KH, H], BF16)
            icWT_sb = pre.tile([128, KH, H], BF16)
            ihb_sb = pre.tile([128, KH, BL], F32)
            icb_sb = pre.tile([128, KH, BL], F32)
            bcomb_sb = pre.tile([128, GH, 1], F32)
            meanT_bf = pre.tile([128, KH, BL], BF16)

            nc.sync.dma_start(featT_sb[:], featT_p[:])
            nc.sync.dma_start(embT_sb[:], embT_p[:])
            nc.sync.dma_start(WfT_sb[:], WfT_p[:])
            nc.sync.dma_start(WembT_sb[:], WembT_p[:])
            nc.sync.dma_start(ihWT_sb[:], ihWT_p[:])
            nc.sync.dma_start(icWT_sb[:], icWT_p[:])
            nc.sync.dma_start(ihb_sb[:], ihb_p[:])
            nc.sync.dma_start(icb_sb[:], icb_p[:])
            nc.sync.dma_start(bcomb_sb[:], bcomb_p[:])

            meanT_f = pre.tile([128, KH, BL], F32)
            nc.vector.tensor_reduce(
                meanT_f[:], featT_sb[:], axis=mybir.AxisListType.X, op=ALU.add
            )
            nc.vector.tensor_copy(meanT_bf[:], meanT_f[:])

            # H0 / C0 (2/P folded into ihWT/icWT host-side)
            psum_h0 = prepsum.tile([128, KH, BL], F32)
            psum_c0 = prepsum.tile([128, KH, BL], F32)
            for mh in range(KH):
                for kh in range(KH):
                    nc.tensor.matmul(
                        psum_h0[:, mh, :],
                        ihWT_sb[:, kh, mh * 128:(mh + 1) * 128],
                        meanT_bf[:, kh, :],
                        start=(kh == 0), stop=(kh == KH - 1),
                    )
            for mh in range(KH):
                for kh in range(KH):
                    nc.tensor.matmul(
                        psum_c0[:, mh, :],
                        icWT_sb[:, kh, mh * 128:(mh + 1) * 128],
                        meanT_bf[:, kh, :],
                        start=(kh == 0), stop=(kh == KH - 1),
                    )
            nc.vector.tensor_add(h0_sb[:], psum_h0[:], ihb_sb[:])
            nc.vector.tensor_add(cT[:], psum_c0[:], icb_sb[:])

            # featproT = Wf @ features^T
            for mh in range(KH):
                for half in range(2):
                    psum_fp = prepsum.tile([128, 2, P], F32)
                    for kh in range(KH):
                        nc.tensor.matmul(
                            psum_fp[:],
                            WfT_sb[:, kh, mh * 128:(mh + 1) * 128],
                            featT_sb[:, kh, 2 * half:2 * half + 2, :],
                            start=(kh == 0), stop=(kh == KH - 1),
                        )
                    nc.vector.tensor_copy(
                        featproT[:, mh, 2 * half:2 * half + 2, :], psum_fp[:]
                    )

            # embprojT = Wemb @ emb^T + (b_ih + b_hh)
            embprojT_v = embprojT.rearrange("p gh b t -> p gh (b t)")
            for mh in range(GH):
                psum_ep = prepsum.tile([128, BL * T], F32)
                for kh in range(KH):
                    nc.tensor.matmul(
                        psum_ep[:],
                        WembT_sb[:, kh, mh * 128:(mh + 1) * 128],
                        embT_sb[:, kh, :],
                        start=(kh == 0), stop=(kh == KH - 1),
                    )
                nc.vector.tensor_add(
                    embprojT_v[:, mh, :], psum_ep[:],
                    bcomb_sb[:, mh, :].broadcast_to([128, BL * T]),
                )

        # ---- recurrence ----
        psum = ctx.enter_context(tc.tile_pool(name="psum", bufs=1, space="PSUM"))
        for t in range(T):
            def hsrc(kh):
                if t == 0:
                    return h0_sb[:, kh, :]
                return h_histT[:, kh, :, t - 1]
            # hWh^T [h_out, b] -- head of the per-step critical chain
            psum_hwh = psum.tile([128, KH, BL], F32, tag="hwh")
            for mh in range(KH):
                for kh in range(KH):
                    nc.tensor.matmul(
                        psum_hwh[:, mh, :],
                        WhT_sb[:, kh, mh * 128:(mh + 1) * 128],
                        hsrc(kh),
                        start=(kh == 0), stop=(kh == KH - 1),
                    )
            bias_sb = work.tile([128, KH, BL], F32, tag="bias")
            nc.vector.tensor_add(bias_sb[:], psum_hwh[:], attnb_sb[:])

            # energy = tanh(featproT + bias); scores = v . energy
            psum_sc = psum.tile([4, P], F32, tag="sc")
            for hh in range(KH):
                energy = work.tile([128, BL, P], BF16, tag=f"en{hh % 2}")
                energy_t = work.tile([128, BL, P], BF16, tag=f"et{hh % 2}")
                for b in range(BL):
                    eng = nc.vector if b < 2 else nc.gpsimd
                    eng.tensor_scalar_add(
                        energy[:, b, :],
                        featproT[:, hh, b, :],
                        bias_sb[:, hh, b:b + 1],
                    )
                nc.scalar.activation(energy_t[:], energy[:], AF.Tanh)
                for b in range(BL):
                    nc.tensor.matmul(
                        psum_sc[0:4, :],
                        vmaskT_sb[:, hh, b, :],
                        energy_t[:, b, :],
                        start=(hh == 0 and b == 0),
                        stop=(hh == KH - 1 and b == BL - 1),
                    )

            # gates h-part: off-chain, fills PE idle during softmax window
            psum_gh = psum.tile([128, GH, BL], F32, tag="gh")
            for mh in range(GH):
                for kh in range(KH, 2 * KH):
                    nc.tensor.matmul(
                        psum_gh[:, mh, :],
                        WcombT_sb[:, kh, mh * 128:(mh + 1) * 128],
                        hsrc(kh - KH),
                        start=(kh == KH), stop=(kh == 2 * KH - 1),
                    )

            # softmax over p (scores are small; no max-subtraction)
            esc = work.tile([4, P], F32, tag="esc")
            esum = work.tile([4, 1], F32, tag="esum")
            rsum = work.tile([4, 1], F32, tag="rsum")
            alpha = work.tile([4, P], BF16, tag="alpha")
            nc.scalar.activation(
                esc[0:4, :], psum_sc[0:4, :], AF.Exp, accum_out=esum[0:4, :]
            )
            nc.vector.reciprocal(rsum[0:4, :], esum[0:4, :])
            nc.vector.tensor_scalar_mul(alpha[0:4, :], esc[0:4, :], rsum[0:4, :])

            # alpha^T via PE transpose, then ctx^T directly per (b, eh)
            psum_aT = psum.tile([128, 2, BL], BF16, tag="aT")
            nc.tensor.transpose(psum_aT[:, 0, :], alpha[0:4, 0:PC0], eye4_sb[:])
            nc.tensor.transpose(psum_aT[0:PC1, 1, :], alpha[0:4, PC0:P], eye4_sb[:])
            alphaT_sb = work.tile([128, 2, BL], BF16, tag="alphaT")
            nc.vector.tensor_copy(alphaT_sb[:], psum_aT[:])

            psum_ctxT = psum.tile([128, KH, BL], F32, tag="ctxT")
            for b in range(BL):
                for eh in range(KH):
                    nc.tensor.matmul(
                        psum_ctxT[:, eh, b:b + 1],
                        feat_sb[0:128, b, 0, eh * 128:(eh + 1) * 128],
                        alphaT_sb[0:128, 0, b:b + 1],
                        start=True, stop=False,
                    )
                    nc.tensor.matmul(
                        psum_ctxT[:, eh, b:b + 1],
                        feat_sb[0:PC1, b, 1, eh * 128:(eh + 1) * 128],
                        alphaT_sb[0:PC1, 1, b:b + 1],
                        start=False, stop=True,
                    )
            nc.vector.tensor_copy(ctxhT[:], psum_ctxT[:])

            # gates ctx-part
            psum_g = psum.tile([128, GH, BL], F32, tag="g")
            for mh in range(GH):
                for kh in range(KH):
                    nc.tensor.matmul(
                        psum_g[:, mh, :],
                        WcombT_sb[:, kh, mh * 128:(mh + 1) * 128],
                        ctxhT[:, kh, :],
                        start=(kh == 0), stop=(kh == KH - 1),
                    )
            # pair embproj with the late (ctx) psum so neither TT waits
            # ahead of ready work in the DVE stream
            gates_cb = work.tile([128, GH, BL], F32, tag="gcb")
            nc.vector.tensor_add(gates_cb[:], psum_g[:], embprojT[:, :, :, t])
            gates_sb = work.tile([128, GH, BL], F32, tag="gates")
            nc.vector.tensor_add(gates_sb[:], psum_gh[:], gates_cb[:])

            # LSTM pointwise; gate chunk order [g,f,i,o]; state C=2c, H=2h
            # sigma(x) = 0.5*tanh(x/2) + 0.5 folded via pre-doubled state
            tg = work.tile([128, KH, BL], F32, tag="tg")
            tfio = work.tile([128, 3 * KH, BL], F32, tag="tfio")
            nc.scalar.activation(tg[:], gates_sb[:, 0:KH, :], AF.Tanh)
            nc.scalar.activation(
                tfio[:], gates_sb[:, KH:4 * KH, :], AF.Tanh, scale=0.5
            )
            sa = work.tile([128, KH, BL], F32, tag="sa")
            sb_ = work.tile([128, KH, BL], F32, tag="sb")
            # sa = (tanh(f/2)+1)*C ; sb = (tanh(i/2)+1)*tanh(g)
            nc.vector.scalar_tensor_tensor(
                sa[:], tfio[:, 0:KH, :], 1.0, cT[:], op0=ALU.add, op1=ALU.mult
            )
            nc.vector.scalar_tensor_tensor(
                sb_[:], tfio[:, KH:2 * KH, :], 1.0, tg[:],
                op0=ALU.add, op1=ALU.mult,
            )
            # C' = 0.5*sa + sb
            nc.vector.scalar_tensor_tensor(
                cT[:], sa[:], 0.5, sb_[:], op0=ALU.mult, op1=ALU.add
            )
            tc_ = work.tile([128, KH, BL], F32, tag="tc")
            nc.scalar.activation(tc_[:], cT[:], AF.Tanh, scale=0.5)
            # H = (tanh(o/2)+1)*tanh(c)  [= 2h]
            nc.vector.scalar_tensor_tensor(
                h_histT[:, :, :, t], tfio[:, 2 * KH:3 * KH, :], 1.0, tc_[:],
                op0=ALU.add, op1=ALU.mult,
            )

        # ---- phase 2: gather H, vocab-sharded projection ----
        with (
            tc.tile_pool(name="ph2", bufs=2) as ph2,
            tc.tile_pool(name="ph2psum", bufs=2, space="PSUM") as ph2psum,
        ):
            nc.sync.dma_start(linWT_sb[:], linWT_p[:])
            nc.sync.dma_start(linb_sb[:], linb_p[:])
            h_flat = h_histT.rearrange("p kh b t -> p (kh b t)")
            if n_cores > 1:
                nc.sync.dma_start(hbounce[:], h_flat[:])
                nc.gpsimd.collective_compute(
                    "AllGather",
                    ALU.bypass,
                    replica_groups=[list(range(n_cores))],
                    ins=[hbounce[:]],
                    outs=[hgath[:]],
                )
            NCH = VS // 500
            for r in range(n_cores):
                if n_cores > 1:
                    hall = ph2.tile([128, KH * BL * T], BF16)
                    nc.sync.dma_start(
                        hall[:], hgath[r * 128:(r + 1) * 128, :]
                    )
                    hall_v = hall.rearrange("p (kh m) -> p kh m", kh=KH)
                else:
                    hall_v = h_histT.rearrange("p kh b t -> p kh (b t)")
                out_sb = ph2.tile([128, VS], BF16)
                for nch in range(NCH):
                    psum_o = ph2psum.tile([128, 500], F32)
                    for kh in range(KH):
                        nc.tensor.matmul(
                            psum_o[:],
                            hall_v[:, kh, :],
                            linWT_sb[:, kh, nch * 500:(nch + 1) * 500],
                            start=(kh == 0), stop=(kh == KH - 1),
                        )
                    nc.vector.tensor_add(
                        out_sb[:, nch * 500:(nch + 1) * 500],
                        psum_o[:],
                        linb_sb[:, nch * 500:(nch + 1) * 500],
                    )
                nc.sync.dma_start(out_p[r * 128:(r + 1) * 128, :], out_sb[:])

    nc.compile()
    return nc


def make_in_maps(inputs, n_cores):
    f32 = np.float32
    feats = np.asarray(inputs["features"], f32)          # [B, P, E]
    caps = np.asarray(inputs["captions"]).astype(np.int64)
    embW = np.asarray(inputs["embed_W"], f32)
    attnW = np.asarray(inputs["attn_W"], f32)
    attnb = np.asarray(inputs["attn_b"], f32)
    vw = np.asarray(inputs["v_w"], f32)
    Wih = np.asarray(inputs["W_ih"], f32)
    Whh = np.asarray(inputs["W_hh"], f32)
    bih = np.asarray(inputs["b_ih"], f32)
    bhh = np.asarray(inputs["b_hh"], f32)
    linW = np.asarray(inputs["lin_W"], f32)
    linb = np.asarray(inputs["lin_b"], f32)
    ihW = np.asarray(inputs["inith_W"], f32)
    ihb = np.asarray(inputs["inith_b"], f32)
    icW = np.asarray(inputs["initc_W"], f32)
    icb = np.asarray(inputs["initc_b"], f32)

    Wf, Wh = attnW[:, :E], attnW[:, E:]
    Wemb, Wctx = Wih[:, :E], Wih[:, E:]

    # gate rows permuted (i,f,g,o) -> (g,f,i,o)
    def gperm(m):
        return np.concatenate(
            [m[2 * H:3 * H], m[H:2 * H], m[0:H], m[3 * H:4 * H]], axis=0
        )

    # h stored as 2h -> h-consuming weights x0.5
    Wcomb = gperm(np.concatenate([Wctx, 0.5 * Whh], axis=1))  # [4H, E+H]
    Wemb_p = gperm(Wemb)
    bcomb_v = gperm((bih + bhh).reshape(-1, 1)).reshape(-1)

    def bft(m):  # [K, M] fp32 -> [128, K//128, M] bf16 tiles
        return np.ascontiguousarray(_to_tiles(m)).astype(bf16)

    WfT_h = bft(Wf.T)
    WhT_h = bft(0.5 * Wh.T)
    WcombT_h = bft(Wcomb.T)
    WembT_h = bft(Wemb_p.T)
    ihWT_h = bft(2.0 * ihW.T / P)
    icWT_h = bft(2.0 * icW.T / P)

    def pexp(vec, reps):  # [D] -> [128, D//128, reps] f32
        return np.repeat(
            vec.reshape(-1, 128).T[:, :, None], reps, axis=2
        ).astype(f32)

    ihb_h = pexp(2.0 * ihb, BL)
    icb_h = pexp(2.0 * icb, BL)
    attnb_h = pexp(attnb, BL)
    bcomb_h = pexp(bcomb_v, 1)
    eye4_h = np.eye(4, dtype=bf16)

    vmask = np.zeros((128, KH, BL, BL), np.float32)
    vt = vw.reshape(KH, 128).T                            # [128, KH]
    for b in range(BL):
        vmask[:, :, b, b] = vt
    vmask_h = vmask.astype(bf16)

    in_maps = []
    for k in range(n_cores):
        b0 = k * BL
        fk = feats[b0:b0 + BL]                            # [BL, P, E]
        featT = (
            fk.transpose(2, 0, 1)
            .reshape(KH, 128, BL, P)
            .transpose(1, 0, 2, 3)
        )
        featpad = np.zeros((BL, 2, 128, E), f32)
        featpad[:, 0] = fk[:, 0:128]
        featpad[:, 1, 0:PC1] = fk[:, 128:P]
        feat_h = featpad.transpose(2, 0, 1, 3)            # [128, BL, 2, E]
        embk = embW[caps[b0:b0 + BL]]                     # [BL, T, E]
        embT = (
            embk.transpose(2, 0, 1)
            .reshape(KH, 128, BL * T)
            .transpose(1, 0, 2)
        )
        linWT_k = _to_tiles(0.5 * linW[k * VS:(k + 1) * VS].T)
        linb_k = np.repeat(
            linb[k * VS:(k + 1) * VS][None, :], 128, axis=0
        ).astype(f32)
        in_maps.append({
            "featT": np.ascontiguousarray(featT).astype(bf16),
            "feat": np.ascontiguousarray(feat_h).astype(bf16),
            "embT": np.ascontiguousarray(embT).astype(bf16),
            "linWT": np.ascontiguousarray(linWT_k).astype(bf16),
            "linb": linb_k,
            "WfT": WfT_h, "WhT": WhT_h, "WcombT": WcombT_h, "WembT": WembT_h,
            "ihWT": ihWT_h, "icWT": icWT_h,
            "ihb": ihb_h, "icb": icb_h, "attnb": attnb_h, "bcomb": bcomb_h,
            "vmaskT": vmask_h, "eye4": eye4_h,
        })
    return in_maps


def unshard(results, n_cores):
    # each core's "out": [n_cores*BL*T, VS] rows ordered (rank, b_local, t)
    shards = [
        np.asarray(results[k]["out"]).astype(np.float32).reshape(
            n_cores * BL, T, VS
        )
        for k in range(n_cores)
    ]
    return np.concatenate(shards, axis=-1).reshape(B, T, V)


_NC_CACHE = {}


def kernel(**inputs):
    n_cores = N_CORES
    if n_cores not in _NC_CACHE:
        _NC_CACHE[n_cores] = build_nc(n_cores)
    nc = _NC_CACHE[n_cores]
    in_maps = make_in_maps(inputs, n_cores)
    res = run_bass_kernel_spmd(nc, in_maps, list(range(n_cores)))
    return unshard(res.results, n_cores)


if __name__ == "__main__":
    import reference
    inputs = reference.setup_inputs()
    out = kernel(**{k: np.asarray(v) for k, v in inputs.items()})
    print(out.shape, out.dtype)


# revision 18
# speedup vs baseline: 2.1126x; 2.1126x over previous
"""CaptionDecoder Trainium2 kernel: 8-core SPMD.

Strategy:
  - Recurrence (attention + LSTM, T=32 steps) is batch-sharded: each core
    owns 4 of 32 batch rows; all weights replicated. No collectives.
  - Hidden states for all steps are then AllGathered (1MB, one collective)
    and the vocab projection [1024,512]@[512,32000] is vocab-sharded:
    each core computes a 4000-wide vocab slice for the full batch.
  - All matmul operands are bf16 (fp32 PSUM accumulation).
  - Per-step critical path minimized:
      * energy bias-add via tensor_scalar (per-partition scalar) instead
        of broadcast tensor_tensor
      * softmax uses real Exp (+accum_out running sum); LSTM sigmoids are
        computed as 0.5*tanh(x/2)+0.5 so every ACT call stays in the
        exp_and_others table set (tanh+exp) -> zero table reloads
      * state is kept pre-doubled (H=2h, C=2c) with compensating 0.5x in
        the h-consuming weights so the sigmoid affine folds into
        scalar_tensor_tensor ops
      * ctx^T computed directly via per-(b,eh) matmuls with features as
        the stationary operand (no [4,E] detour / masked copies)
      * gate matmuls accumulate h-part and ctx-part into one PSUM group
  - Output written bf16 (host upcasts): halves the 16MB/core output DMA.
"""
import sys
import numpy as np
import ml_dtypes

sys.path.insert(0, "/opt/trn_rl_repo")

from contextlib import ExitStack

import concourse.bass as bass
import concourse.tile as tile
from concourse import bacc, mybir
from concourse.bass_utils import run_bass_kernel_spmd

BF16 = mybir.dt.bfloat16
F32 = mybir.dt.float32
AF = mybir.ActivationFunctionType
ALU = mybir.AluOpType

E = 512
H = 512
V = 32000
B = 32
P = 196
T = 32
N_CORES = 8
BL = B // N_CORES          # 4 batch rows per core
VS = V // N_CORES          # 4000 vocab per core
KH = 4                     # 128-chunks of E / H
GH = 16                    # 128-chunks of 4H
PC0, PC1 = 128, P - 128    # pixel chunks 128 + 68

bf16 = ml_dtypes.bfloat16


def _to_tiles(mat_T):
    """[K, M] -> [128, K//128, M] (partition-major K tiles)."""
    Kdim, M = mat_T.shape
    return mat_T.reshape(Kdim // 128, 128, M).transpose(1, 0, 2)


def build_nc(n_cores):
    nc = bacc.Bacc(
        "TRN2",
        target_bir_lowering=False,
        debug=False,
        enable_asserts=False,
        num_devices=n_cores,
    )

    def inp(name, shape, dt=BF16):
        return nc.declare_dram_parameter(name, list(shape), dt, isOutput=False).ap()

    # Per-core sharded inputs
    featT_p = inp("featT", [128, KH, BL, P])            # features^T [el, eh, b, p]
    feat_p = inp("feat", [128, BL, 2, E])               # [p_lo, b, pc, e] (pc1 rows>=68 pad)
    embT_p = inp("embT", [128, KH, BL * T])             # gathered emb^T [el, eh, (b,t)]
    linWT_p = inp("linWT", [128, KH, VS])               # lin_W shard^T (x0.5)
    linb_p = inp("linb", [128, VS], F32)                # host-expanded over partitions
    # Replicated weights (gate rows permuted to [g,f,i,o]; h-consumers x0.5)
    WfT_p = inp("WfT", [128, KH, H])
    WhT_p = inp("WhT", [128, KH, H])
    WcombT_p = inp("WcombT", [128, 2 * KH, 4 * H])      # [ctx;h] -> gates
    WembT_p = inp("WembT", [128, KH, 4 * H])
    ihWT_p = inp("ihWT", [128, KH, H])                  # inith_W^T * 2/P
    icWT_p = inp("icWT", [128, KH, H])
    ihb_p = inp("ihb", [128, KH, BL], F32)              # x2
    icb_p = inp("icb", [128, KH, BL], F32)              # x2
    attnb_p = inp("attnb", [128, KH, BL], F32)
    bcomb_p = inp("bcomb", [128, GH, 1], F32)           # (b_ih+b_hh) permuted
    vmaskT_p = inp("vmaskT", [128, KH, BL, BL])         # v masked per batch col
    eye4_p = inp("eye4", [4, 4])

    out_p = nc.declare_dram_parameter(
        "out", [n_cores * BL * T, VS], BF16, isOutput=True
    ).ap()

    NCHUNK = 4
    CS = KH * BL * (T // NCHUNK)
    hbounce = [
        nc.dram_tensor(f"hbounce{c}", [128, CS], BF16).ap()
        for c in range(NCHUNK)
    ]
    hgath = [
        nc.dram_tensor(
            f"hgath{c}", [n_cores * 128, CS], BF16, addr_space="Shared"
        ).ap()
        for c in range(NCHUNK)
    ]

    with tile.TileContext(nc) as tc, ExitStack() as ctx:
        const = ctx.enter_context(tc.tile_pool(name="const", bufs=1))
        state = ctx.enter_context(tc.tile_pool(name="state", bufs=1))
        work = ctx.enter_context(tc.tile_pool(name="work", bufs=2))

        # ---- persistent SBUF ----
        feat_sb = const.tile([128, BL, 2, E], BF16)
        WhT_sb = const.tile([128, KH, H], BF16)
        WcombT_sb = const.tile([128, 2 * KH, 4 * H], BF16)
        vmaskT_sb = const.tile([128, KH, BL, BL], BF16)
        eye4_sb = const.tile([4, 4], BF16)
        attnb_sb = const.tile([128, KH, BL], F32)
        featproT = const.tile([128, KH, BL, P], BF16)   # Wf@features^T
        embprojT = const.tile([128, GH, BL, T], F32)    # Wemb@emb^T + b (permuted)
        linWT_sb = const.tile([128, KH, VS], BF16)
        linb_sb = const.tile([128, VS], F32)

        ctxhT = state.tile([128, KH, BL], BF16)         # ctx^T gate input
        h0_sb = state.tile([128, KH, BL], BF16)         # H0 = 2h0
        cT = state.tile([128, KH, BL], F32)             # C = 2c
        h_histT = state.tile([128, NCHUNK, KH, BL, T // NCHUNK], BF16)  # H=2h
        hbounce_sb = state.tile([128, KH * BL * T], BF16)

        # (input DMAs are emitted in first-use order below)

        # ---- precompute ----
        with (
            tc.tile_pool(name="pre", bufs=1) as pre,
            tc.tile_pool(name="prepsum", bufs=1, space="PSUM") as prepsum,
        ):
            featT_sb = pre.tile([128, KH, BL, P], BF16)
            embT_sb = pre.tile([128, KH, BL * T], BF16)
            WfT_sb = pre.tile([128, KH, H], BF16)
            WembT_sb = pre.tile([128, KH, 4 * H], BF16)
            ihWT_sb = pre.tile([128, KH, H], BF16)
            icWT_sb = pre.tile([128, KH, H], BF16)
            ihb_sb = pre.tile([128, KH, BL], F32)
            icb_sb = pre.tile([128, KH, BL], F32)
            bcomb_sb = pre.tile([128, GH, 1], F32)
            meanT_bf = pre.tile([128, KH, BL], BF16)

            nc.sync.dma_start(featT_sb[:], featT_p[:])
            nc.sync.dma_start(WfT_sb[:], WfT_p[:])
            nc.sync.dma_start(attnb_sb[:], attnb_p[:])
            nc.sync.dma_start(ihWT_sb[:], ihWT_p[:])
            nc.sync.dma_start(icWT_sb[:], icWT_p[:])
            nc.sync.dma_start(ihb_sb[:], ihb_p[:])
            nc.sync.dma_start(icb_sb[:], icb_p[:])
            nc.sync.dma_start(embT_sb[:], embT_p[:])
            nc.sync.dma_start(WembT_sb[:], WembT_p[:])
            nc.sync.dma_start(bcomb_sb[:], bcomb_p[:])
            nc.sync.dma_start(WhT_sb[:], WhT_p[:])
            nc.sync.dma_start(vmaskT_sb[:], vmaskT_p[:])
            nc.sync.dma_start(eye4_sb[:], eye4_p[:])
            nc.sync.dma_start(feat_sb[:], feat_p[:])
            nc.sync.dma_start(WcombT_sb[:], WcombT_p[:])

            meanT_f = pre.tile([128, KH, BL], F32)
            nc.vector.tensor_reduce(
                meanT_f[:], featT_sb[:], axis=mybir.AxisListType.X, op=ALU.add
            )
            nc.vector.tensor_copy(meanT_bf[:], meanT_f[:])

            # H0 / C0 (2/P folded into ihWT/icWT host-side)
            psum_h0 = prepsum.tile([128, KH, BL], F32)
            psum_c0 = prepsum.tile([128, KH, BL], F32)
            for mh in range(KH):
                for kh in range(KH):
                    nc.tensor.matmul(
                        psum_h0[:, mh, :],
                        ihWT_sb[:, kh, mh * 128:(mh + 1) * 128],
                        meanT_bf[:, kh, :],
                        start=(kh == 0), stop=(kh == KH - 1),
                    )
            for mh in range(KH):
                for kh in range(KH):
                    nc.tensor.matmul(
                        psum_c0[:, mh, :],
                        icWT_sb[:, kh, mh * 128:(mh + 1) * 128],
                        meanT_bf[:, kh, :],
                        start=(kh == 0), stop=(kh == KH - 1),
                    )
            nc.vector.tensor_add(h0_sb[:], psum_h0[:], ihb_sb[:])
            nc.vector.tensor_add(cT[:], psum_c0[:], icb_sb[:])

            # featproT = Wf @ features^T
            for mh in range(KH):
                for half in range(2):
                    psum_fp = prepsum.tile([128, 2, P], F32, bufs=2)
                    for kh in range(KH):
                        nc.tensor.matmul(
                            psum_fp[:],
                            WfT_sb[:, kh, mh * 128:(mh + 1) * 128],
                            featT_sb[:, kh, 2 * half:2 * half + 2, :],
                            start=(kh == 0), stop=(kh == KH - 1),
                        )
                    for bb in range(2):
                        b = 2 * half + bb
                        nc.vector.tensor_scalar_add(
                            featproT[:, mh, b, :],
                            psum_fp[:, bb, :],
                            attnb_sb[:, mh, b:b + 1],
                        )

            # embprojT = Wemb @ emb^T + (b_ih + b_hh)
            embprojT_v = embprojT.rearrange("p gh b t -> p gh (b t)")
            for mh in range(GH):
                psum_ep = prepsum.tile([128, BL * T], F32, bufs=2)
                for kh in range(KH):
                    nc.tensor.matmul(
                        psum_ep[:],
                        WembT_sb[:, kh, mh * 128:(mh + 1) * 128],
                        embT_sb[:, kh, :],
                        start=(kh == 0), stop=(kh == KH - 1),
                    )
                nc.vector.tensor_add(
                    embprojT_v[:, mh, :], psum_ep[:],
                    bcomb_sb[:, mh, :].broadcast_to([128, BL * T]),
                )

        # ---- recurrence ----
        rec = ExitStack()
        psum = rec.enter_context(tc.tile_pool(name="psum", bufs=1, space="PSUM"))
        for t in range(T):
            TC = T // NCHUNK
            def hsrc(kh):
                if t == 0:
                    return h0_sb[:, kh, :]
                return h_histT[:, (t - 1) // TC, kh, :, (t - 1) % TC]
            # hWh^T [h_out, b] -- head of the per-step critical chain;
            # split across two PSUM banks so the first energy chunk's bias
            # is available after only half the matmuls
            psum_hwh0 = psum.tile([128, 2, BL], F32, tag="hwh0")
            psum_hwh1 = psum.tile([128, 2, BL], F32, tag="hwh1")
            psum_hwh = [psum_hwh0, psum_hwh1]
            bias_sb = work.tile([128, KH, BL], F32, tag="bias")
            for half in range(2):
                for m2 in range(2):
                    for kh in range(KH):
                        nc.tensor.matmul(
                            psum_hwh[half][:, m2, :],
                            WhT_sb[:, kh,
                                   (2 * half + m2) * 128:
                                   (2 * half + m2 + 1) * 128],
                            hsrc(kh),
                            start=(kh == 0), stop=(kh == KH - 1),
                        )
                nc.vector.tensor_copy(
                    bias_sb[:, 2 * half:2 * half + 2, :], psum_hwh[half][:]
                )
            # energy = tanh(featproT + bias); scores = v . energy
            psum_sc = psum.tile([4, P], F32, tag="sc")
            for hh in range(KH):
                energy = work.tile([128, BL, P], BF16, tag=f"en{hh % 2}")
                energy_t = work.tile([128, BL, P], BF16, tag=f"et{hh % 2}")
                for b in range(BL):
                    nc.vector.tensor_scalar_add(
                        energy[:, b, :],
                        featproT[:, hh, b, :],
                        bias_sb[:, hh, b:b + 1],
                    )
                nc.scalar.activation(energy_t[:], energy[:], AF.Tanh)
                for b in range(BL):
                    nc.tensor.matmul(
                        psum_sc[0:4, :],
                        vmaskT_sb[:, hh, b, :],
                        energy_t[:, b, :],
                        start=(hh == 0 and b == 0),
                        stop=(hh == KH - 1 and b == BL - 1),
                    )

            # gates h-part: off-chain, fills PE idle during softmax window
            psum_gh = psum.tile([128, GH, BL], F32, tag="gh")
            for mh in range(GH):
                for kh in range(KH, 2 * KH):
                    nc.tensor.matmul(
                        psum_gh[:, mh, :],
                        WcombT_sb[:, kh, mh * 128:(mh + 1) * 128],
                        hsrc(kh - KH),
                        start=(kh == KH), stop=(kh == 2 * KH - 1),
                    )

            # softmax over p (scores are small; no max-subtraction)
            esc = work.tile([4, P], F32, tag="esc")
            esum = work.tile([4, 1], F32, tag="esum")
            rsum = work.tile([4, 1], F32, tag="rsum")
            alpha = work.tile([4, P], BF16, tag="alpha")
            nc.scalar.activation(
                esc[0:4, :], psum_sc[0:4, :], AF.Exp, accum_out=esum[0:4, :]
            )
            nc.vector.reciprocal(rsum[0:4, :], esum[0:4, :])
            nc.vector.tensor_scalar_mul(alpha[0:4, :], esc[0:4, :], rsum[0:4, :])
            psum_aT = psum.tile([128, 2, BL], BF16, tag="aT")
            nc.tensor.transpose(psum_aT[:, 0, :], alpha[0:4, 0:PC0], eye4_sb[:])
            nc.tensor.transpose(psum_aT[0:PC1, 1, :], alpha[0:4, PC0:P], eye4_sb[:])
            alphaT_sb = work.tile([128, 2, BL], BF16, tag="alphaT")
            nc.vector.tensor_copy(alphaT_sb[:], psum_aT[:])

            psum_ctxT = psum.tile([128, KH, BL], F32, tag="ctxT")
            for eh in range(KH):
                for b in range(BL):
                    nc.tensor.matmul(
                        psum_ctxT[:, eh, b:b + 1],
                        feat_sb[0:128, b, 0, eh * 128:(eh + 1) * 128],
                        alphaT_sb[0:128, 0, b:b + 1],
                        start=True, stop=False,
                    )
                    nc.tensor.matmul(
                        psum_ctxT[:, eh, b:b + 1],
                        feat_sb[0:PC1, b, 1, eh * 128:(eh + 1) * 128],
                        alphaT_sb[0:PC1, 1, b:b + 1],
                        start=False, stop=True,
                    )
            nc.vector.tensor_copy(ctxhT[:], psum_ctxT[:])

            # gates ctx-part
            psum_g = psum.tile([128, GH, BL], F32, tag="g")
            for mh in range(GH):
                for kh in range(KH):
                    nc.tensor.matmul(
                        psum_g[:, mh, :],
                        WcombT_sb[:, kh, mh * 128:(mh + 1) * 128],
                        ctxhT[:, kh, :],
                        start=(kh == 0), stop=(kh == KH - 1),
                    )
            # pair embproj with the late (ctx) psum so neither TT waits
            # ahead of ready work in the DVE stream
            gates_cb = work.tile([128, GH, BL], F32, tag="gcb")
            nc.vector.tensor_add(gates_cb[:], psum_g[:], embprojT[:, :, :, t])
            gates_sb = work.tile([128, GH, BL], F32, tag="gates")
            nc.vector.tensor_add(gates_sb[:], psum_gh[:], gates_cb[:])

            # LSTM pointwise; gate chunk order [g,f,i,o]; state C=2c, H=2h
            # sigma(x) = 0.5*tanh(x/2) + 0.5 folded via pre-doubled state
            tg = work.tile([128, KH, BL], F32, tag="tg")
            tfio = work.tile([128, 3 * KH, BL], F32, tag="tfio")
            nc.scalar.activation(tg[:], gates_sb[:, 0:KH, :], AF.Tanh)
            nc.scalar.activation(
                tfio[:], gates_sb[:, KH:4 * KH, :], AF.Tanh, scale=0.5
            )
            sa = work.tile([128, KH, BL], F32, tag="sa")
            sb_ = work.tile([128, KH, BL], F32, tag="sb")
            # sa = (tanh(f/2)+1)*C ; sb = (tanh(i/2)+1)*tanh(g)
            nc.vector.scalar_tensor_tensor(
                sa[:], tfio[:, 0:KH, :], 1.0, cT[:], op0=ALU.add, op1=ALU.mult
            )
            nc.vector.scalar_tensor_tensor(
                sb_[:], tfio[:, KH:2 * KH, :], 1.0, tg[:],
                op0=ALU.add, op1=ALU.mult,
            )
            # C' = 0.5*sa + sb
            nc.vector.scalar_tensor_tensor(
                cT[:], sa[:], 0.5, sb_[:], op0=ALU.mult, op1=ALU.add
            )
            tc_ = work.tile([128, KH, BL], F32, tag="tc")
            nc.scalar.activation(tc_[:], cT[:], AF.Tanh, scale=0.5)
            # H = (tanh(o/2)+1)*tanh(c)  [= 2h]
            nc.vector.scalar_tensor_tensor(
                h_histT[:, t // TC, :, :, t % TC], tfio[:, 2 * KH:3 * KH, :],
                1.0, tc_[:],
                op0=ALU.add, op1=ALU.mult,
            )
            if t % TC == TC - 1 and n_cores > 1:
                c = t // TC
                nc.sync.dma_start(
                    hbounce[c][:],
                    h_histT[:, c].rearrange("p kh b tc -> p (kh b tc)"),
                )
                nc.gpsimd.collective_compute(
                    "AllGather",
                    ALU.bypass,
                    replica_groups=[list(range(n_cores))],
                    ins=[hbounce[c][:]],
                    outs=[hgath[c][:]],
                )

        rec.close()

        # ---- phase 2: gather H, vocab-sharded projection ----
        with (
            tc.tile_pool(name="ph2", bufs=2) as ph2,
            tc.tile_pool(name="ph2psum", bufs=4, space="PSUM") as ph2psum,
        ):
            nc.sync.dma_start(linWT_sb[:], linWT_p[:])
            nc.sync.dma_start(linb_sb[:], linb_p[:])
            NCH = VS // 500
            for r in range(n_cores):
                hall = ph2.tile([128, KH, NCHUNK, BL * (T // NCHUNK)], BF16)
                for c in range(NCHUNK):
                    nc.sync.dma_start(
                        hall[:, :, c, :],
                        hgath[c][r * 128:(r + 1) * 128, :].rearrange(
                            "r (kh m) -> r kh m", kh=KH
                        ),
                    )
                # rows of out: M = (c, b, tc) contiguous per kh
                hall_m = hall.rearrange("p kh c m -> p kh (c m)")
                out_sb = ph2.tile([128, VS], BF16)
                for nch in range(NCH):
                    psum_o = ph2psum.tile([128, 500], F32)
                    for kh in range(KH):
                        nc.tensor.matmul(
                            psum_o[:],
                            hall_m[:, kh, :],
                            linWT_sb[:, kh, nch * 500:(nch + 1) * 500],
                            start=(kh == 0), stop=(kh == KH - 1),
                        )
                    nc.vector.tensor_add(
                        out_sb[:, nch * 500:(nch + 1) * 500],
                        psum_o[:],
                        linb_sb[:, nch * 500:(nch + 1) * 500],
                    )
                nc.sync.dma_start(out_p[r * 128:(r + 1) * 128, :], out_sb[:])

    nc.compile()
    return nc


def make_in_maps(inputs, n_cores):
    f32 = np.float32
    feats = np.asarray(inputs["features"], f32)          # [B, P, E]
    caps = np.asarray(inputs["captions"]).astype(np.int64)
    embW = np.asarray(inputs["embed_W"], f32)
    attnW = np.asarray(inputs["attn_W"], f32)
    attnb = np.asarray(inputs["attn_b"], f32)
    vw = np.asarray(inputs["v_w"], f32)
    Wih = np.asarray(inputs["W_ih"], f32)
    Whh = np.asarray(inputs["W_hh"], f32)
    bih = np.asarray(inputs["b_ih"], f32)
    bhh = np.asarray(inputs["b_hh"], f32)
    linW = np.asarray(inputs["lin_W"], f32)
    linb = np.asarray(inputs["lin_b"], f32)
    ihW = np.asarray(inputs["inith_W"], f32)
    ihb = np.asarray(inputs["inith_b"], f32)
    icW = np.asarray(inputs["initc_W"], f32)
    icb = np.asarray(inputs["initc_b"], f32)

    Wf, Wh = attnW[:, :E], attnW[:, E:]
    Wemb, Wctx = Wih[:, :E], Wih[:, E:]

    # gate rows permuted (i,f,g,o) -> (g,f,i,o)
    def gperm(m):
        return np.concatenate(
            [m[2 * H:3 * H], m[H:2 * H], m[0:H], m[3 * H:4 * H]], axis=0
        )

    # h stored as 2h -> h-consuming weights x0.5
    Wcomb = gperm(np.concatenate([Wctx, 0.5 * Whh], axis=1))  # [4H, E+H]
    Wemb_p = gperm(Wemb)
    bcomb_v = gperm((bih + bhh).reshape(-1, 1)).reshape(-1)

    def bft(m):  # [K, M] fp32 -> [128, K//128, M] bf16 tiles
        return np.ascontiguousarray(_to_tiles(m)).astype(bf16)

    WfT_h = bft(Wf.T)
    WhT_h = bft(0.5 * Wh.T)
    WcombT_h = bft(Wcomb.T)
    WembT_h = bft(Wemb_p.T)
    ihWT_h = bft(2.0 * ihW.T / P)
    icWT_h = bft(2.0 * icW.T / P)

    def pexp(vec, reps):  # [D] -> [128, D//128, reps] f32
        return np.repeat(
            vec.reshape(-1, 128).T[:, :, None], reps, axis=2
        ).astype(f32)

    ihb_h = pexp(2.0 * ihb, BL)
    icb_h = pexp(2.0 * icb, BL)
    attnb_h = pexp(attnb, BL)
    bcomb_h = pexp(bcomb_v, 1)
    eye4_h = np.eye(4, dtype=bf16)

    vmask = np.zeros((128, KH, BL, BL), np.float32)
    vt = vw.reshape(KH, 128).T                            # [128, KH]
    for b in range(BL):
        vmask[:, :, b, b] = vt
    vmask_h = vmask.astype(bf16)

    in_maps = []
    for k in range(n_cores):
        b0 = k * BL
        fk = feats[b0:b0 + BL]                            # [BL, P, E]
        featT = (
            fk.transpose(2, 0, 1)
            .reshape(KH, 128, BL, P)
            .transpose(1, 0, 2, 3)
        )
        featpad = np.zeros((BL, 2, 128, E), f32)
        featpad[:, 0] = fk[:, 0:128]
        featpad[:, 1, 0:PC1] = fk[:, 128:P]
        feat_h = featpad.transpose(2, 0, 1, 3)            # [128, BL, 2, E]
        embk = embW[caps[b0:b0 + BL]]                     # [BL, T, E]
        embT = (
            embk.transpose(2, 0, 1)
            .reshape(KH, 128, BL * T)
            .transpose(1, 0, 2)
        )
        linWT_k = _to_tiles(0.5 * linW[k * VS:(k + 1) * VS].T)
        linb_k = np.repeat(
            linb[k * VS:(k + 1) * VS][None, :], 128, axis=0
        ).astype(f32)
        in_maps.append({
            "featT": np.ascontiguousarray(featT).astype(bf16),
            "feat": np.ascontiguousarray(feat_h).astype(bf16),
            "embT": np.ascontiguousarray(embT).astype(bf16),
            "linWT": np.ascontiguousarray(linWT_k).astype(bf16),
            "linb": linb_k,
            "WfT": WfT_h, "WhT": WhT_h, "WcombT": WcombT_h, "WembT": WembT_h,
            "ihWT": ihWT_h, "icWT": icWT_h,
            "ihb": ihb_h, "icb": icb_h, "attnb": attnb_h, "bcomb": bcomb_h,
            "vmaskT": vmask_h, "eye4": eye4_h,
        })
    return in_maps


def unshard(results, n_cores):
    # each core's "out": [n_cores*BL*T, VS] rows ordered (rank, b_local, t)
    shards = [
        np.asarray(results[k]["out"]).astype(np.float32)
        .reshape(n_cores, 4, BL, T // 4, VS)
        .transpose(0, 2, 1, 3, 4)
        .reshape(n_cores * BL, T, VS)
        for k in range(n_cores)
    ]
    return np.concatenate(shards, axis=-1).reshape(B, T, V)


_NC_CACHE = {}


def kernel(**inputs):
    n_cores = N_CORES
    if n_cores not in _NC_CACHE:
        _NC_CACHE[n_cores] = build_nc(n_cores)
    nc = _NC_CACHE[n_cores]
    in_maps = make_in_maps(inputs, n_cores)
    res = run_bass_kernel_spmd(nc, in_maps, list(range(n_cores)))
    return unshard(res.results, n_cores)


if __name__ == "__main__":
    import reference
    inputs = reference.setup_inputs()
    out = kernel(**{k: np.asarray(v) for k, v in inputs.items()})
    print(out.shape, out.dtype)


# revision 20
# speedup vs baseline: 2.2341x; 1.0575x over previous
"""CaptionDecoder Trainium2 kernel: 8-core SPMD.

Strategy:
  - Recurrence (attention + LSTM, T=32 steps) is batch-sharded: each core
    owns 4 of 32 batch rows; all weights replicated. No collectives.
  - Hidden states for all steps are then AllGathered (1MB, one collective)
    and the vocab projection [1024,512]@[512,32000] is vocab-sharded:
    each core computes a 4000-wide vocab slice for the full batch.
  - All matmul operands are bf16 (fp32 PSUM accumulation).
  - Per-step critical path minimized:
      * energy bias-add via tensor_scalar (per-partition scalar) instead
        of broadcast tensor_tensor
      * softmax uses real Exp (+accum_out running sum); LSTM sigmoids are
        computed as 0.5*tanh(x/2)+0.5 so every ACT call stays in the
        exp_and_others table set (tanh+exp) -> zero table reloads
      * state is kept pre-doubled (H=2h, C=2c) with compensating 0.5x in
        the h-consuming weights so the sigmoid affine folds into
        scalar_tensor_tensor ops
      * ctx^T computed directly via per-(b,eh) matmuls with features as
        the stationary operand (no [4,E] detour / masked copies)
      * gate matmuls accumulate h-part and ctx-part into one PSUM group
  - Output written bf16 (host upcasts): halves the 16MB/core output DMA.
"""
import sys
import numpy as np
import ml_dtypes

sys.path.insert(0, "/opt/trn_rl_repo")

from contextlib import ExitStack

import concourse.bass as bass
import concourse.tile as tile
from concourse import bacc, mybir
from concourse.bass_utils import run_bass_kernel_spmd

BF16 = mybir.dt.bfloat16
F32 = mybir.dt.float32
AF = mybir.ActivationFunctionType
ALU = mybir.AluOpType

E = 512
H = 512
V = 32000
B = 32
P = 196
T = 32
N_CORES = 8
BL = B // N_CORES          # 4 batch rows per core
VS = V // N_CORES          # 4000 vocab per core
KH = 4                     # 128-chunks of E / H
GH = 16                    # 128-chunks of 4H
PC0, PC1 = 128, P - 128    # pixel chunks 128 + 68

bf16 = ml_dtypes.bfloat16


def _to_tiles(mat_T):
    """[K, M] -> [128, K//128, M] (partition-major K tiles)."""
    Kdim, M = mat_T.shape
    return mat_T.reshape(Kdim // 128, 128, M).transpose(1, 0, 2)


def build_nc(n_cores):
    nc = bacc.Bacc(
        "TRN2",
        target_bir_lowering=False,
        debug=False,
        enable_asserts=False,
        num_devices=n_cores,
    )

    def inp(name, shape, dt=BF16):
        return nc.declare_dram_parameter(name, list(shape), dt, isOutput=False).ap()

    # Per-core sharded inputs
    featT_p = inp("featT", [128, KH, BL, P])            # features^T [el, eh, b, p]
    feat_p = inp("feat", [128, BL, 2, E])               # [p_lo, b, pc, e] (pc1 rows>=68 pad)
    embT_p = inp("embT", [128, KH, BL * T])             # gathered emb^T [el, eh, (b,t)]
    linWT_p = inp("linWT", [128, KH, VS])               # lin_W shard^T (x0.5)
    linb_p = inp("linb", [128, VS], F32)                # host-expanded over partitions
    # Replicated weights (gate rows permuted to [g,f,i,o]; h-consumers x0.5)
    WfT_p = inp("WfT", [128, KH, H])
    WhT_p = inp("WhT", [128, KH, H])
    WcombT_p = inp("WcombT", [128, 2 * KH, 4 * H])      # [ctx;h] -> gates
    WembT_p = inp("WembT", [128, KH, 4 * H])
    ihWT_p = inp("ihWT", [128, KH, H])                  # inith_W^T * 2/P
    icWT_p = inp("icWT", [128, KH, H])
    ihb_p = inp("ihb", [128, KH, BL], F32)              # x2
    icb_p = inp("icb", [128, KH, BL], F32)              # x2
    attnb_p = inp("attnb", [128, KH, BL], F32)
    bcomb_p = inp("bcomb", [128, GH, 1], F32)           # (b_ih+b_hh) permuted
    vmaskT_p = inp("vmaskT", [128, KH, BL, BL])         # v masked per batch col
    eye4_p = inp("eye4", [4, 4])

    out_p = nc.declare_dram_parameter(
        "out", [n_cores * BL * T, VS], BF16, isOutput=True
    ).ap()

    NCHUNK = 4
    CS = KH * BL * (T // NCHUNK)
    hbounce = [
        nc.dram_tensor(f"hbounce{c}", [128, CS], BF16).ap()
        for c in range(NCHUNK)
    ]
    hgath = [
        nc.dram_tensor(
            f"hgath{c}", [n_cores * 128, CS], BF16, addr_space="Shared"
        ).ap()
        for c in range(NCHUNK)
    ]

    with tile.TileContext(nc) as tc, ExitStack() as ctx:
        const = ctx.enter_context(tc.tile_pool(name="const", bufs=1))
        state = ctx.enter_context(tc.tile_pool(name="state", bufs=1))
        work = ctx.enter_context(tc.tile_pool(name="work", bufs=2))

        # ---- persistent SBUF ----
        feat_sb = const.tile([128, BL, 2, E], BF16)
        WhT_sb = const.tile([128, KH, H], BF16)
        WcombT_sb = const.tile([128, 2 * KH, 4 * H], BF16)
        vmaskT_sb = const.tile([128, KH, BL, BL], BF16)
        eye4_sb = const.tile([4, 4], BF16)
        attnb_sb = const.tile([128, KH, BL], F32)
        featproT = const.tile([128, KH, BL, P], BF16)   # Wf@features^T
        embprojT = const.tile([128, GH, BL, T], F32)    # Wemb@emb^T + b (permuted)
        linWT_sb = const.tile([128, KH, VS], BF16)
        linb_sb = const.tile([128, VS], F32)

        ctxhT = state.tile([128, KH, BL], BF16)         # ctx^T gate input
        h0_sb = state.tile([128, KH, BL], BF16)         # H0 = 2h0
        cT = state.tile([128, KH, BL], F32)             # C = 2c
        h_histT = state.tile([128, NCHUNK, KH, BL, T // NCHUNK], BF16)  # H=2h
        hbounce_sb = state.tile([128, KH * BL * T], BF16)

        # (input DMAs are emitted in first-use order below)

        # ---- precompute ----
        with (
            tc.tile_pool(name="pre", bufs=1) as pre,
            tc.tile_pool(name="prepsum", bufs=1, space="PSUM") as prepsum,
        ):
            featT_sb = pre.tile([128, KH, BL, P], BF16)
            embT_sb = pre.tile([128, KH, BL * T], BF16)
            WfT_sb = pre.tile([128, KH, H], BF16)
            WembT_sb = pre.tile([128, KH, 4 * H], BF16)
            ihWT_sb = pre.tile([128, KH, H], BF16)
            icWT_sb = pre.tile([128, KH, H], BF16)
            ihb_sb = pre.tile([128, KH, BL], F32)
            icb_sb = pre.tile([128, KH, BL], F32)
            bcomb_sb = pre.tile([128, GH, 1], F32)
            meanT_bf = pre.tile([128, KH, BL], BF16)

            nc.sync.dma_start(featT_sb[:], featT_p[:])
            nc.sync.dma_start(WfT_sb[:], WfT_p[:])
            nc.sync.dma_start(attnb_sb[:], attnb_p[:])
            nc.sync.dma_start(ihWT_sb[:], ihWT_p[:])
            nc.sync.dma_start(icWT_sb[:], icWT_p[:])
            nc.sync.dma_start(ihb_sb[:], ihb_p[:])
            nc.sync.dma_start(icb_sb[:], icb_p[:])
            nc.sync.dma_start(embT_sb[:], embT_p[:])
            nc.sync.dma_start(WembT_sb[:], WembT_p[:])
            nc.sync.dma_start(bcomb_sb[:], bcomb_p[:])
            nc.sync.dma_start(WhT_sb[:], WhT_p[:])
            nc.sync.dma_start(vmaskT_sb[:], vmaskT_p[:])
            nc.sync.dma_start(eye4_sb[:], eye4_p[:])
            nc.sync.dma_start(feat_sb[:], feat_p[:])
            nc.sync.dma_start(WcombT_sb[:], WcombT_p[:])

            meanT_f = pre.tile([128, KH, BL], F32)
            nc.vector.tensor_reduce(
                meanT_f[:], featT_sb[:], axis=mybir.AxisListType.X, op=ALU.add
            )
            nc.vector.tensor_copy(meanT_bf[:], meanT_f[:])

            # H0 / C0 (2/P folded into ihWT/icWT host-side)
            psum_h0 = prepsum.tile([128, KH, BL], F32)
            psum_c0 = prepsum.tile([128, KH, BL], F32)
            for mh in range(KH):
                for kh in range(KH):
                    nc.tensor.matmul(
                        psum_h0[:, mh, :],
                        ihWT_sb[:, kh, mh * 128:(mh + 1) * 128],
                        meanT_bf[:, kh, :],
                        start=(kh == 0), stop=(kh == KH - 1),
                    )
            for mh in range(KH):
                for kh in range(KH):
                    nc.tensor.matmul(
                        psum_c0[:, mh, :],
                        icWT_sb[:, kh, mh * 128:(mh + 1) * 128],
                        meanT_bf[:, kh, :],
                        start=(kh == 0), stop=(kh == KH - 1),
                    )
            nc.vector.tensor_add(h0_sb[:], psum_h0[:], ihb_sb[:])
            nc.vector.tensor_add(cT[:], psum_c0[:], icb_sb[:])

            # featproT = Wf @ features^T
            for mh in range(KH):
                for half in range(2):
                    psum_fp = prepsum.tile([128, 2, P], F32, bufs=2)
                    for kh in range(KH):
                        nc.tensor.matmul(
                            psum_fp[:],
                            WfT_sb[:, kh, mh * 128:(mh + 1) * 128],
                            featT_sb[:, kh, 2 * half:2 * half + 2, :],
                            start=(kh == 0), stop=(kh == KH - 1),
                        )
                    for bb in range(2):
                        b = 2 * half + bb
                        nc.vector.tensor_scalar_add(
                            featproT[:, mh, b, :],
                            psum_fp[:, bb, :],
                            attnb_sb[:, mh, b:b + 1],
                        )

            # embprojT = Wemb @ emb^T + (b_ih + b_hh)
            embprojT_v = embprojT.rearrange("p gh b t -> p gh (b t)")
            for mh in range(GH):
                psum_ep = prepsum.tile([128, BL * T], F32, bufs=2)
                for kh in range(KH):
                    nc.tensor.matmul(
                        psum_ep[:],
                        WembT_sb[:, kh, mh * 128:(mh + 1) * 128],
                        embT_sb[:, kh, :],
                        start=(kh == 0), stop=(kh == KH - 1),
                    )
                nc.vector.tensor_add(
                    embprojT_v[:, mh, :], psum_ep[:],
                    bcomb_sb[:, mh, :].broadcast_to([128, BL * T]),
                )

        # ---- recurrence ----
        rec = ExitStack()
        psum = rec.enter_context(tc.tile_pool(name="psum", bufs=1, space="PSUM"))
        for t in range(T):
            TC = T // NCHUNK
            def hsrc(kh):
                if t == 0:
                    return h0_sb[:, kh, :]
                return h_histT[:, (t - 1) // TC, kh, :, (t - 1) % TC]
            # hWh^T [h_out, b] -- head of the per-step critical chain;
            # split across two PSUM banks so the first energy chunk's bias
            # is available after only half the matmuls
            psum_hwh0 = psum.tile([128, 2, BL], F32, tag="hwh0")
            psum_hwh1 = psum.tile([128, 2, BL], F32, tag="hwh1")
            psum_hwh = [psum_hwh0, psum_hwh1]
            bias_sb = work.tile([128, KH, BL], F32, tag="bias")
            for half in range(2):
                for m2 in range(2):
                    for kh in range(KH):
                        nc.tensor.matmul(
                            psum_hwh[half][:, m2, :],
                            WhT_sb[:, kh,
                                   (2 * half + m2) * 128:
                                   (2 * half + m2 + 1) * 128],
                            hsrc(kh),
                            start=(kh == 0), stop=(kh == KH - 1),
                        )
                nc.vector.tensor_copy(
                    bias_sb[:, 2 * half:2 * half + 2, :], psum_hwh[half][:]
                )
            # energy = tanh(featproT + bias); scores = v . energy
            psum_sc = psum.tile([4, P], F32, tag="sc")
            for hh in range(KH):
                energy = work.tile([128, BL, P], BF16, tag=f"en{hh % 2}")
                energy_t = work.tile([128, BL, P], BF16, tag=f"et{hh % 2}")
                for b in range(BL):
                    nc.vector.tensor_scalar_add(
                        energy[:, b, :],
                        featproT[:, hh, b, :],
                        bias_sb[:, hh, b:b + 1],
                    )
                nc.scalar.activation(energy_t[:], energy[:], AF.Tanh)
                for b in range(BL):
                    nc.tensor.matmul(
                        psum_sc[0:4, :],
                        vmaskT_sb[:, hh, b, :],
                        energy_t[:, b, :],
                        start=(hh == 0 and b == 0),
                        stop=(hh == KH - 1 and b == BL - 1),
                    )

            # gates h-part: off-chain, fills PE idle during softmax window
            psum_gh = psum.tile([128, GH, BL], F32, tag="gh")
            for mh in range(GH):
                for kh in range(KH, 2 * KH):
                    nc.tensor.matmul(
                        psum_gh[:, mh, :],
                        WcombT_sb[:, kh, mh * 128:(mh + 1) * 128],
                        hsrc(kh - KH),
                        start=(kh == KH), stop=(kh == 2 * KH - 1),
                    )

            # softmax over p (scores are small; no max-subtraction)
            esc = work.tile([4, P], F32, tag="esc")
            esum = work.tile([4, 1], F32, tag="esum")
            rsum = work.tile([4, 1], F32, tag="rsum")
            alpha = work.tile([4, P], BF16, tag="alpha")
            nc.scalar.activation(
                esc[0:4, :], psum_sc[0:4, :], AF.Exp, accum_out=esum[0:4, :]
            )
            nc.vector.reciprocal(rsum[0:4, :], esum[0:4, :])
            nc.vector.tensor_scalar_mul(alpha[0:4, :], esc[0:4, :], rsum[0:4, :])
            psum_aT = psum.tile([128, 2, BL], BF16, tag="aT")
            nc.tensor.transpose(psum_aT[:, 0, :], alpha[0:4, 0:PC0], eye4_sb[:])
            nc.tensor.transpose(psum_aT[0:PC1, 1, :], alpha[0:4, PC0:P], eye4_sb[:])
            alphaT_sb = work.tile([128, 2, BL], BF16, tag="alphaT")
            nc.vector.tensor_copy(alphaT_sb[:], psum_aT[:])

            psum_ctxT = psum.tile([128, KH, BL], F32, tag="ctxT")
            for eh in range(KH):
                for b in range(BL):
                    nc.tensor.matmul(
                        psum_ctxT[:, eh, b:b + 1],
                        feat_sb[0:128, b, 0, eh * 128:(eh + 1) * 128],
                        alphaT_sb[0:128, 0, b:b + 1],
                        start=True, stop=False,
                    )
                    nc.tensor.matmul(
                        psum_ctxT[:, eh, b:b + 1],
                        feat_sb[0:PC1, b, 1, eh * 128:(eh + 1) * 128],
                        alphaT_sb[0:PC1, 1, b:b + 1],
                        start=False, stop=True,
                    )
            nc.vector.tensor_copy(ctxhT[:], psum_ctxT[:])

            # gates ctx-part
            psum_g = psum.tile([128, GH, BL], F32, tag="g")
            for mh in range(GH):
                for kh in range(KH):
                    nc.tensor.matmul(
                        psum_g[:, mh, :],
                        WcombT_sb[:, kh, mh * 128:(mh + 1) * 128],
                        ctxhT[:, kh, :],
                        start=(kh == 0), stop=(kh == KH - 1),
                    )
            # pair embproj with the late (ctx) psum so neither TT waits
            # ahead of ready work in the DVE stream
            gates_cb = work.tile([128, GH, BL], F32, tag="gcb")
            nc.vector.tensor_add(gates_cb[:], psum_g[:], embprojT[:, :, :, t])
            gates_sb = work.tile([128, GH, BL], F32, tag="gates")
            nc.vector.tensor_add(gates_sb[:], psum_gh[:], gates_cb[:])

            # LSTM pointwise; gate chunk order [g,f,i,o]; state C=2c, H=2h
            # sigma(x) = 0.5*tanh(x/2) + 0.5 folded via pre-doubled state
            tg = work.tile([128, KH, BL], F32, tag="tg")
            tfio = work.tile([128, 3 * KH, BL], F32, tag="tfio")
            nc.scalar.activation(tg[:], gates_sb[:, 0:KH, :], AF.Tanh)
            nc.scalar.activation(
                tfio[:], gates_sb[:, KH:4 * KH, :], AF.Tanh, scale=0.5
            )
            sa = work.tile([128, KH, BL], F32, tag="sa")
            sb_ = work.tile([128, KH, BL], F32, tag="sb")
            # sa = (tanh(f/2)+1)*C ; sb = (tanh(i/2)+1)*tanh(g)
            nc.vector.scalar_tensor_tensor(
                sa[:], tfio[:, 0:KH, :], 1.0, cT[:], op0=ALU.add, op1=ALU.mult
            )
            nc.vector.scalar_tensor_tensor(
                sb_[:], tfio[:, KH:2 * KH, :], 1.0, tg[:],
                op0=ALU.add, op1=ALU.mult,
            )
            # C' = 0.5*sa + sb
            nc.vector.scalar_tensor_tensor(
                cT[:], sa[:], 0.5, sb_[:], op0=ALU.mult, op1=ALU.add
            )
            tc_ = work.tile([128, KH, BL], F32, tag="tc")
            nc.scalar.activation(tc_[:], cT[:], AF.Tanh, scale=0.5)
            # H = (tanh(o/2)+1)*tanh(c)  [= 2h]
            nc.vector.scalar_tensor_tensor(
                h_histT[:, t // TC, :, :, t % TC], tfio[:, 2 * KH:3 * KH, :],
                1.0, tc_[:],
                op0=ALU.add, op1=ALU.mult,
            )
            if t % TC == TC - 1 and n_cores > 1:
                c = t // TC
                nc.sync.dma_start(
                    hbounce[c][:],
                    h_histT[:, c].rearrange("p kh b tc -> p (kh b tc)"),
                )
                nc.gpsimd.collective_compute(
                    "AllGather",
                    ALU.bypass,
                    replica_groups=[list(range(n_cores))],
                    ins=[hbounce[c][:]],
                    outs=[hgath[c][:]],
                )

        rec.close()

        # ---- phase 2: gather H, vocab-sharded projection ----
        with (
            tc.tile_pool(name="ph2", bufs=2) as ph2,
            tc.tile_pool(name="ph2psum", bufs=4, space="PSUM") as ph2psum,
        ):
            nc.sync.dma_start(linWT_sb[:], linWT_p[:])
            nc.sync.dma_start(linb_sb[:], linb_p[:])
            NCH = VS // 500
            for r in range(n_cores):
                hall = ph2.tile([128, KH, NCHUNK, BL * (T // NCHUNK)], BF16)
                for c in range(NCHUNK):
                    nc.sync.dma_start(
                        hall[:, :, c, :],
                        hgath[c][r * 128:(r + 1) * 128, :].rearrange(
                            "r (kh m) -> r kh m", kh=KH
                        ),
                    )
                # rows of out: M = (c, b, tc) contiguous per kh
                hall_m = hall.rearrange("p kh c m -> p kh (c m)")
                out_sb = ph2.tile([128, VS], BF16)
                for nch in range(NCH):
                    psum_o = ph2psum.tile([128, 500], F32)
                    for kh in range(KH):
                        nc.tensor.matmul(
                            psum_o[:],
                            hall_m[:, kh, :],
                            linWT_sb[:, kh, nch * 500:(nch + 1) * 500],
                            start=(kh == 0), stop=(kh == KH - 1),
                        )
                    nc.vector.tensor_add(
                        out_sb[:, nch * 500:(nch + 1) * 500],
                        psum_o[:],
                        linb_sb[:, nch * 500:(nch + 1) * 500],
                    )
                nc.sync.dma_start(out_p[r * 128:(r + 1) * 128, :], out_sb[:])

    nc.compile()
    return nc


def make_in_maps(inputs, n_cores):
    f32 = np.float32
    feats = np.asarray(inputs["features"], f32)          # [B, P, E]
    caps = np.asarray(inputs["captions"]).astype(np.int64)
    embW = np.asarray(inputs["embed_W"], f32)
    attnW = np.asarray(inputs["attn_W"], f32)
    attnb = np.asarray(inputs["attn_b"], f32)
    vw = np.asarray(inputs["v_w"], f32)
    Wih = np.asarray(inputs["W_ih"], f32)
    Whh = np.asarray(inputs["W_hh"], f32)
    bih = np.asarray(inputs["b_ih"], f32)
    bhh = np.asarray(inputs["b_hh"], f32)
    linW = np.asarray(inputs["lin_W"], f32)
    linb = np.asarray(inputs["lin_b"], f32)
    ihW = np.asarray(inputs["inith_W"], f32)
    ihb = np.asarray(inputs["inith_b"], f32)
    icW = np.asarray(inputs["initc_W"], f32)
    icb = np.asarray(inputs["initc_b"], f32)

    Wf, Wh = attnW[:, :E], attnW[:, E:]
    Wemb, Wctx = Wih[:, :E], Wih[:, E:]

    # gate rows permuted (i,f,g,o) -> (g,f,i,o)
    def gperm(m):
        return np.concatenate(
            [m[2 * H:3 * H], m[H:2 * H], m[0:H], m[3 * H:4 * H]], axis=0
        )

    # h stored as 2h -> h-consuming weights x0.5
    Wcomb = gperm(np.concatenate([Wctx, 0.5 * Whh], axis=1))  # [4H, E+H]
    Wemb_p = gperm(Wemb)
    bcomb_v = gperm((bih + bhh).reshape(-1, 1)).reshape(-1)

    def bft(m):  # [K, M] fp32 -> [128, K//128, M] bf16 tiles
        return np.ascontiguousarray(_to_tiles(m)).astype(bf16)

    WfT_h = bft(Wf.T)
    WhT_h = bft(0.5 * Wh.T)
    WcombT_h = bft(Wcomb.T)
    WembT_h = bft(Wemb_p.T)
    ihWT_h = bft(2.0 * ihW.T / P)
    icWT_h = bft(2.0 * icW.T / P)

    def pexp(vec, reps):  # [D] -> [128, D//128, reps] f32
        return np.repeat(
            vec.reshape(-1, 128).T[:, :, None], reps, axis=2
        ).astype(f32)

    ihb_h = pexp(2.0 * ihb, BL)
    icb_h = pexp(2.0 * icb, BL)
    attnb_h = pexp(attnb, BL)
    bcomb_h = pexp(bcomb_v, 1)
    eye4_h = np.eye(4, dtype=bf16)

    vmask = np.zeros((128, KH, BL, BL), np.float32)
    vt = vw.reshape(KH, 128).T                            # [128, KH]
    for b in range(BL):
        vmask[:, :, b, b] = vt
    vmask_h = vmask.astype(bf16)

    in_maps = []
    for k in range(n_cores):
        b0 = k * BL
        fk = feats[b0:b0 + BL]                            # [BL, P, E]
        featT = (
            fk.transpose(2, 0, 1)
            .reshape(KH, 128, BL, P)
            .transpose(1, 0, 2, 3)
        )
        featpad = np.zeros((BL, 2, 128, E), f32)
        featpad[:, 0] = fk[:, 0:128]
        featpad[:, 1, 0:PC1] = fk[:, 128:P]
        feat_h = featpad.transpose(2, 0, 1, 3)            # [128, BL, 2, E]
        embk = embW[caps[b0:b0 + BL]]                     # [BL, T, E]
        embT = (
            embk.transpose(2, 0, 1)
            .reshape(KH, 128, BL * T)
            .transpose(1, 0, 2)
        )
        linWT_k = _to_tiles(0.5 * linW[k * VS:(k + 1) * VS].T)
        linb_k = np.repeat(
            linb[k * VS:(k + 1) * VS][None, :], 128, axis=0
        ).astype(f32)
        in_maps.append({
            "featT": np.ascontiguousarray(featT).astype(bf16),
            "feat": np.ascontiguousarray(feat_h).astype(bf16),
            "embT": np.ascontiguousarray(embT).astype(bf16),
            "linWT": np.ascontiguousarray(linWT_k).astype(bf16),
            "linb": linb_k,
            "WfT": WfT_h, "WhT": WhT_h, "WcombT": WcombT_h, "WembT": WembT_h,
            "ihWT": ihWT_h, "icWT": icWT_h,
            "ihb": ihb_h, "icb": icb_h, "attnb": attnb_h, "bcomb": bcomb_h,
            "vmaskT": vmask_h, "eye4": eye4_h,
        })
    return in_maps


def unshard(results, n_cores):
    # each core's "out": [n_cores*BL*T, VS] rows ordered (rank, b_local, t)
    shards = [
        np.asarray(results[k]["out"]).astype(np.float32)
        .reshape(n_cores, 4, BL, T // 4, VS)
        .transpose(0, 2, 1, 3, 4)
        .reshape(n_cores * BL, T, VS)
        for k in range(n_cores)
    ]
    return np.concatenate(shards, axis=-1).reshape(B, T, V)


_NC_CACHE = {}


def kernel(**inputs):
    n_cores = N_CORES
    if n_cores not in _NC_CACHE:
        _NC_CACHE[n_cores] = build_nc(n_cores)
    nc = _NC_CACHE[n_cores]
    in_maps = make_in_maps(inputs, n_cores)
    res = run_bass_kernel_spmd(nc, in_maps, list(range(n_cores)))
    return unshard(res.results, n_cores)


if __name__ == "__main__":
    import reference
    inputs = reference.setup_inputs()
    out = kernel(**{k: np.asarray(v) for k, v in inputs.items()})
    print(out.shape, out.dtype)


# revision 21
# speedup vs baseline: 2.2646x; 1.0136x over previous
"""CaptionDecoder Trainium2 kernel: 8-core SPMD.

Strategy:
  - Recurrence (attention + LSTM, T=32 steps) is batch-sharded: each core
    owns 4 of 32 batch rows; all weights replicated. No collectives.
  - Hidden states for all steps are then AllGathered (1MB, one collective)
    and the vocab projection [1024,512]@[512,32000] is vocab-sharded:
    each core computes a 4000-wide vocab slice for the full batch.
  - All matmul operands are bf16 (fp32 PSUM accumulation).
  - Per-step critical path minimized:
      * energy bias-add via tensor_scalar (per-partition scalar) instead
        of broadcast tensor_tensor
      * softmax uses real Exp (+accum_out running sum); LSTM sigmoids are
        computed as 0.5*tanh(x/2)+0.5 so every ACT call stays in the
        exp_and_others table set (tanh+exp) -> zero table reloads
      * state is kept pre-doubled (H=2h, C=2c) with compensating 0.5x in
        the h-consuming weights so the sigmoid affine folds into
        scalar_tensor_tensor ops
      * ctx^T computed directly via per-(b,eh) matmuls with features as
        the stationary operand (no [4,E] detour / masked copies)
      * gate matmuls accumulate h-part and ctx-part into one PSUM group
  - Output written bf16 (host upcasts): halves the 16MB/core output DMA.
"""
import sys
import numpy as np
import ml_dtypes

sys.path.insert(0, "/opt/trn_rl_repo")

from contextlib import ExitStack

import concourse.bass as bass
import concourse.tile as tile
from concourse import bacc, mybir
from concourse.bass_utils import run_bass_kernel_spmd

BF16 = mybir.dt.bfloat16
F32 = mybir.dt.float32
AF = mybir.ActivationFunctionType
ALU = mybir.AluOpType

E = 512
H = 512
V = 32000
B = 32
P = 196
T = 32
N_CORES = 8
BL = B // N_CORES          # 4 batch rows per core
VS = V // N_CORES          # 4000 vocab per core
KH = 4                     # 128-chunks of E / H
GH = 16                    # 128-chunks of 4H
PC0, PC1 = 128, P - 128    # pixel chunks 128 + 68

bf16 = ml_dtypes.bfloat16


def _to_tiles(mat_T):
    """[K, M] -> [128, K//128, M] (partition-major K tiles)."""
    Kdim, M = mat_T.shape
    return mat_T.reshape(Kdim // 128, 128, M).transpose(1, 0, 2)


def build_nc(n_cores):
    nc = bacc.Bacc(
        "TRN2",
        target_bir_lowering=False,
        debug=False,
        enable_asserts=False,
        num_devices=n_cores,
    )

    def inp(name, shape, dt=BF16):
        return nc.declare_dram_parameter(name, list(shape), dt, isOutput=False).ap()

    # Per-core sharded inputs
    featT_p = inp("featT", [128, KH, BL, P])            # features^T [el, eh, b, p]
    feat_p = inp("feat", [128, BL, 2, E])               # [p_lo, b, pc, e] (pc1 rows>=68 pad)
    embT_p = inp("embT", [128, KH, BL * T])             # gathered emb^T [el, eh, (b,t)]
    linWT_p = inp("linWT", [128, KH, VS])               # lin_W shard^T (x0.5)
    linb_p = inp("linb", [128, VS], F32)                # host-expanded over partitions
    # Replicated weights (gate rows permuted to [g,f,i,o]; h-consumers x0.5)
    WfT_p = inp("WfT", [128, KH, H])
    WhT_p = inp("WhT", [128, KH, H])
    WcombT_p = inp("WcombT", [128, 2 * KH, 4 * H])      # [ctx;h] -> gates
    WembT_p = inp("WembT", [128, KH, 4 * H])
    ihWT_p = inp("ihWT", [128, KH, H])                  # inith_W^T * 2/P
    icWT_p = inp("icWT", [128, KH, H])
    ihb_p = inp("ihb", [128, KH, BL], F32)              # x2
    icb_p = inp("icb", [128, KH, BL], F32)              # x2
    attnb_p = inp("attnb", [128, KH, BL], F32)
    bcomb_p = inp("bcomb", [128, GH, 1], F32)           # (b_ih+b_hh) permuted
    vmaskT_p = inp("vmaskT", [128, KH, BL, BL])         # v masked per batch col
    eye4_p = inp("eye4", [4, 4])

    out_p = nc.declare_dram_parameter(
        "out", [n_cores * BL * T, VS], BF16, isOutput=True
    ).ap()

    NCHUNK = 4
    CS = KH * BL * (T // NCHUNK)
    hbounce = [
        nc.dram_tensor(f"hbounce{c}", [128, CS], BF16).ap()
        for c in range(NCHUNK)
    ]
    hgath = [
        nc.dram_tensor(
            f"hgath{c}", [n_cores * 128, CS], BF16, addr_space="Shared"
        ).ap()
        for c in range(NCHUNK)
    ]

    with tile.TileContext(nc) as tc, ExitStack() as ctx:
        const = ctx.enter_context(tc.tile_pool(name="const", bufs=1))
        state = ctx.enter_context(tc.tile_pool(name="state", bufs=1))
        work = ctx.enter_context(tc.tile_pool(name="work", bufs=2))

        # ---- persistent SBUF ----
        feat_sb = const.tile([128, BL, 2, E], BF16)
        WhT_sb = const.tile([128, KH, H], BF16)
        WcombT_sb = const.tile([128, 2 * KH, 4 * H], BF16)
        vmaskT_sb = const.tile([128, KH, BL, BL], BF16)
        eye4_sb = const.tile([4, 4], BF16)
        attnb_sb = const.tile([128, KH, BL], F32)
        featproT = const.tile([128, KH, BL, P], BF16)   # Wf@features^T
        embprojT = const.tile([128, GH, BL, T], F32)    # Wemb@emb^T + b (permuted)
        linWT_sb = const.tile([128, KH, VS], BF16)
        linb_sb = const.tile([128, VS], F32)

        ctxhT = state.tile([128, KH, BL], BF16)         # ctx^T gate input
        h0_sb = state.tile([128, KH, BL], BF16)         # H0 = 2h0
        cT = state.tile([128, KH, BL], F32)             # C = 2c
        h_histT = state.tile([128, NCHUNK, KH, BL, T // NCHUNK], BF16)  # H=2h
        hbounce_sb = state.tile([128, KH * BL * T], BF16)

        # (input DMAs are emitted in first-use order below)

        # ---- precompute ----
        with (
            tc.tile_pool(name="pre", bufs=1) as pre,
            tc.tile_pool(name="prepsum", bufs=1, space="PSUM") as prepsum,
        ):
            featT_sb = pre.tile([128, KH, BL, P], BF16)
            embT_sb = pre.tile([128, KH, BL * T], BF16)
            WfT_sb = pre.tile([128, KH, H], BF16)
            WembT_sb = pre.tile([128, KH, 4 * H], BF16)
            ihWT_sb = pre.tile([128, KH, H], BF16)
            icWT_sb = pre.tile([128, KH, H], BF16)
            ihb_sb = pre.tile([128, KH, BL], F32)
            icb_sb = pre.tile([128, KH, BL], F32)
            bcomb_sb = pre.tile([128, GH, 1], F32)
            meanT_bf = pre.tile([128, KH, BL], BF16)

            nc.sync.dma_start(featT_sb[:], featT_p[:])
            nc.sync.dma_start(WfT_sb[:], WfT_p[:])
            nc.sync.dma_start(attnb_sb[:], attnb_p[:])
            nc.sync.dma_start(ihWT_sb[:], ihWT_p[:])
            nc.sync.dma_start(icWT_sb[:], icWT_p[:])
            nc.sync.dma_start(ihb_sb[:], ihb_p[:])
            nc.sync.dma_start(icb_sb[:], icb_p[:])
            nc.sync.dma_start(embT_sb[:], embT_p[:])
            nc.sync.dma_start(WembT_sb[:], WembT_p[:])
            nc.sync.dma_start(bcomb_sb[:], bcomb_p[:])
            nc.sync.dma_start(WhT_sb[:], WhT_p[:])
            nc.sync.dma_start(vmaskT_sb[:], vmaskT_p[:])
            nc.sync.dma_start(eye4_sb[:], eye4_p[:])
            nc.sync.dma_start(feat_sb[:], feat_p[:])
            nc.sync.dma_start(WcombT_sb[:], WcombT_p[:])

            meanT_f = pre.tile([128, KH, BL], F32)
            nc.vector.tensor_reduce(
                meanT_f[:], featT_sb[:], axis=mybir.AxisListType.X, op=ALU.add
            )
            nc.vector.tensor_copy(meanT_bf[:], meanT_f[:])

            # H0 / C0 (2/P folded into ihWT/icWT host-side)
            psum_h0 = prepsum.tile([128, KH, BL], F32)
            psum_c0 = prepsum.tile([128, KH, BL], F32)
            for mh in range(KH):
                for kh in range(KH):
                    nc.tensor.matmul(
                        psum_h0[:, mh, :],
                        ihWT_sb[:, kh, mh * 128:(mh + 1) * 128],
                        meanT_bf[:, kh, :],
                        start=(kh == 0), stop=(kh == KH - 1),
                    )
            for mh in range(KH):
                for kh in range(KH):
                    nc.tensor.matmul(
                        psum_c0[:, mh, :],
                        icWT_sb[:, kh, mh * 128:(mh + 1) * 128],
                        meanT_bf[:, kh, :],
                        start=(kh == 0), stop=(kh == KH - 1),
                    )
            nc.vector.tensor_add(h0_sb[:], psum_h0[:], ihb_sb[:])
            nc.vector.tensor_add(cT[:], psum_c0[:], icb_sb[:])

            # featproT = Wf @ features^T
            for mh in range(KH):
                for half in range(2):
                    psum_fp = prepsum.tile([128, 2, P], F32, bufs=2)
                    for kh in range(KH):
                        nc.tensor.matmul(
                            psum_fp[:],
                            WfT_sb[:, kh, mh * 128:(mh + 1) * 128],
                            featT_sb[:, kh, 2 * half:2 * half + 2, :],
                            start=(kh == 0), stop=(kh == KH - 1),
                        )
                    for bb in range(2):
                        b = 2 * half + bb
                        nc.vector.tensor_scalar_add(
                            featproT[:, mh, b, :],
                            psum_fp[:, bb, :],
                            attnb_sb[:, mh, b:b + 1],
                        )

            # embprojT = Wemb @ emb^T + (b_ih + b_hh)
            embprojT_v = embprojT.rearrange("p gh b t -> p gh (b t)")
            for mh in range(GH):
                psum_ep = prepsum.tile([128, BL * T], F32, bufs=2)
                for kh in range(KH):
                    nc.tensor.matmul(
                        psum_ep[:],
                        WembT_sb[:, kh, mh * 128:(mh + 1) * 128],
                        embT_sb[:, kh, :],
                        start=(kh == 0), stop=(kh == KH - 1),
                    )
                nc.vector.tensor_add(
                    embprojT_v[:, mh, :], psum_ep[:],
                    bcomb_sb[:, mh, :].broadcast_to([128, BL * T]),
                )

        # ---- recurrence ----
        rec = ExitStack()
        psum = rec.enter_context(tc.tile_pool(name="psum", bufs=1, space="PSUM"))
        for t in range(T):
            TC = T // NCHUNK
            def hsrc(kh):
                if t == 0:
                    return h0_sb[:, kh, :]
                return h_histT[:, (t - 1) // TC, kh, :, (t - 1) % TC]
            # hWh^T [h_out, b] -- head of the per-step critical chain;
            # split across two PSUM banks so the first energy chunk's bias
            # is available after only half the matmuls
            psum_hwh0 = psum.tile([128, 2, BL], F32, tag="hwh0")
            psum_hwh1 = psum.tile([128, 2, BL], F32, tag="hwh1")
            psum_hwh = [psum_hwh0, psum_hwh1]
            bias_sb = work.tile([128, KH, BL], F32, tag="bias")
            for half in range(2):
                for m2 in range(2):
                    for kh in range(KH):
                        nc.tensor.matmul(
                            psum_hwh[half][:, m2, :],
                            WhT_sb[:, kh,
                                   (2 * half + m2) * 128:
                                   (2 * half + m2 + 1) * 128],
                            hsrc(kh),
                            start=(kh == 0), stop=(kh == KH - 1),
                        )
                nc.vector.tensor_copy(
                    bias_sb[:, 2 * half:2 * half + 2, :], psum_hwh[half][:]
                )
            # energy = tanh(featproT + bias); scores = v . energy
            psum_sc = psum.tile([4, P], F32, tag="sc")
            for hh in range(KH):
                energy = work.tile([128, BL, P], BF16, tag=f"en{hh % 2}")
                energy_t = work.tile([128, BL, P], BF16, tag=f"et{hh % 2}")
                for b in range(BL):
                    nc.vector.tensor_scalar_add(
                        energy[:, b, :],
                        featproT[:, hh, b, :],
                        bias_sb[:, hh, b:b + 1],
                    )
                nc.scalar.activation(energy_t[:], energy[:], AF.Tanh)
                for b in range(BL):
                    nc.tensor.matmul(
                        psum_sc[0:4, :],
                        vmaskT_sb[:, hh, b, :],
                        energy_t[:, b, :],
                        start=(hh == 0 and b == 0),
                        stop=(hh == KH - 1 and b == BL - 1),
                    )

            # gates h-part: off-chain, fills PE idle during softmax window
            psum_gh = psum.tile([128, GH, BL], F32, tag="gh")
            for mh in range(GH):
                for kh in range(KH, 2 * KH):
                    nc.tensor.matmul(
                        psum_gh[:, mh, :],
                        WcombT_sb[:, kh, mh * 128:(mh + 1) * 128],
                        hsrc(kh - KH),
                        start=(kh == KH), stop=(kh == 2 * KH - 1),
                    )

            # softmax over p (scores are small; no max-subtraction)
            esc = work.tile([4, P], F32, tag="esc")
            esum = work.tile([4, 1], F32, tag="esum")
            rsum = work.tile([4, 1], F32, tag="rsum")
            alpha = work.tile([4, P], BF16, tag="alpha")
            nc.scalar.activation(
                esc[0:4, :], psum_sc[0:4, :], AF.Exp, accum_out=esum[0:4, :]
            )
            nc.vector.reciprocal(rsum[0:4, :], esum[0:4, :])
            nc.vector.tensor_scalar_mul(alpha[0:4, :], esc[0:4, :], rsum[0:4, :])
            psum_aT = psum.tile([128, 2, BL], BF16, tag="aT")
            nc.tensor.transpose(psum_aT[:, 0, :], alpha[0:4, 0:PC0], eye4_sb[:])
            nc.tensor.transpose(psum_aT[0:PC1, 1, :], alpha[0:4, PC0:P], eye4_sb[:])
            alphaT_sb = work.tile([128, 2, BL], BF16, tag="alphaT")
            nc.vector.tensor_copy(alphaT_sb[:], psum_aT[:])

            psum_ctxT = psum.tile([128, KH, BL], F32, tag="ctxT")
            for eh in range(KH):
                for b in range(BL):
                    nc.tensor.matmul(
                        psum_ctxT[:, eh, b:b + 1],
                        feat_sb[0:128, b, 0, eh * 128:(eh + 1) * 128],
                        alphaT_sb[0:128, 0, b:b + 1],
                        start=True, stop=False,
                    )
                    nc.tensor.matmul(
                        psum_ctxT[:, eh, b:b + 1],
                        feat_sb[0:PC1, b, 1, eh * 128:(eh + 1) * 128],
                        alphaT_sb[0:PC1, 1, b:b + 1],
                        start=False, stop=True,
                    )
            nc.vector.tensor_copy(ctxhT[:], psum_ctxT[:])

            # gates ctx-part
            psum_g = psum.tile([128, GH, BL], F32, tag="g")
            for mh in range(GH):
                for kh in range(KH):
                    nc.tensor.matmul(
                        psum_g[:, mh, :],
                        WcombT_sb[:, kh, mh * 128:(mh + 1) * 128],
                        ctxhT[:, kh, :],
                        start=(kh == 0), stop=(kh == KH - 1),
                    )
            # pair embproj with the late (ctx) psum so neither TT waits
            # ahead of ready work in the DVE stream
            gates_cb = work.tile([128, GH, BL], F32, tag="gcb")
            nc.vector.tensor_add(gates_cb[:], psum_g[:], embprojT[:, :, :, t])
            gates_sb = work.tile([128, GH, BL], F32, tag="gates")
            nc.vector.tensor_add(gates_sb[:], psum_gh[:], gates_cb[:])

            # LSTM pointwise; gate chunk order [g,f,i,o]; state C=2c, H=2h
            # sigma(x) = 0.5*tanh(x/2) + 0.5 folded via pre-doubled state
            tall = work.tile([128, 4 * KH, BL], F32, tag="tall")
            nc.scalar.activation(tall[:], gates_sb[:], AF.Tanh)
            sa = work.tile([128, KH, BL], F32, tag="sa")
            sb_ = work.tile([128, KH, BL], F32, tag="sb")
            # sa = (tanh(f/2)+1)*C ; sb = (tanh(i/2)+1)*tanh(g)
            nc.vector.scalar_tensor_tensor(
                sa[:], tall[:, KH:2 * KH, :], 1.0, cT[:],
                op0=ALU.add, op1=ALU.mult,
            )
            nc.vector.scalar_tensor_tensor(
                sb_[:], tall[:, 2 * KH:3 * KH, :], 1.0, tall[:, 0:KH, :],
                op0=ALU.add, op1=ALU.mult,
            )
            # C' = 0.5*sa + sb
            nc.vector.scalar_tensor_tensor(
                cT[:], sa[:], 0.5, sb_[:], op0=ALU.mult, op1=ALU.add
            )
            tc_ = work.tile([128, KH, BL], F32, tag="tc")
            nc.scalar.activation(tc_[:], cT[:], AF.Tanh, scale=0.5)
            # H = (tanh(o/2)+1)*tanh(c)  [= 2h]
            nc.vector.scalar_tensor_tensor(
                h_histT[:, t // TC, :, :, t % TC], tall[:, 3 * KH:4 * KH, :],
                1.0, tc_[:],
                op0=ALU.add, op1=ALU.mult,
            )
            if t % TC == TC - 1 and n_cores > 1:
                c = t // TC
                nc.sync.dma_start(
                    hbounce[c][:],
                    h_histT[:, c].rearrange("p kh b tc -> p (kh b tc)"),
                )
                nc.gpsimd.collective_compute(
                    "AllGather",
                    ALU.bypass,
                    replica_groups=[list(range(n_cores))],
                    ins=[hbounce[c][:]],
                    outs=[hgath[c][:]],
                )

        rec.close()

        # ---- phase 2: gather H, vocab-sharded projection ----
        with (
            tc.tile_pool(name="ph2", bufs=3) as ph2,
            tc.tile_pool(name="ph2psum", bufs=4, space="PSUM") as ph2psum,
        ):
            nc.sync.dma_start(linWT_sb[:], linWT_p[:])
            nc.sync.dma_start(linb_sb[:], linb_p[:])
            NCH = VS // 500
            for r in range(n_cores):
                hall = ph2.tile([128, KH, NCHUNK, BL * (T // NCHUNK)], BF16)
                for c in range(NCHUNK):
                    nc.sync.dma_start(
                        hall[:, :, c, :],
                        hgath[c][r * 128:(r + 1) * 128, :].rearrange(
                            "r (kh m) -> r kh m", kh=KH
                        ),
                    )
                # rows of out: M = (c, b, tc) contiguous per kh
                hall_m = hall.rearrange("p kh c m -> p kh (c m)")
                out_sb = ph2.tile([128, VS], BF16)
                for nch in range(NCH):
                    psum_o = ph2psum.tile([128, 500], F32)
                    for kh in range(KH):
                        nc.tensor.matmul(
                            psum_o[:],
                            hall_m[:, kh, :],
                            linWT_sb[:, kh, nch * 500:(nch + 1) * 500],
                            start=(kh == 0), stop=(kh == KH - 1),
                        )
                    nc.vector.tensor_add(
                        out_sb[:, nch * 500:(nch + 1) * 500],
                        psum_o[:],
                        linb_sb[:, nch * 500:(nch + 1) * 500],
                    )
                nc.sync.dma_start(out_p[r * 128:(r + 1) * 128, :], out_sb[:])

    nc.compile()
    return nc


def make_in_maps(inputs, n_cores):
    f32 = np.float32
    feats = np.asarray(inputs["features"], f32)          # [B, P, E]
    caps = np.asarray(inputs["captions"]).astype(np.int64)
    embW = np.asarray(inputs["embed_W"], f32)
    attnW = np.asarray(inputs["attn_W"], f32)
    attnb = np.asarray(inputs["attn_b"], f32)
    vw = np.asarray(inputs["v_w"], f32)
    Wih = np.asarray(inputs["W_ih"], f32)
    Whh = np.asarray(inputs["W_hh"], f32)
    bih = np.asarray(inputs["b_ih"], f32)
    bhh = np.asarray(inputs["b_hh"], f32)
    linW = np.asarray(inputs["lin_W"], f32)
    linb = np.asarray(inputs["lin_b"], f32)
    ihW = np.asarray(inputs["inith_W"], f32)
    ihb = np.asarray(inputs["inith_b"], f32)
    icW = np.asarray(inputs["initc_W"], f32)
    icb = np.asarray(inputs["initc_b"], f32)

    Wf, Wh = attnW[:, :E], attnW[:, E:]
    Wemb, Wctx = Wih[:, :E], Wih[:, E:]

    # gate rows permuted (i,f,g,o) -> (g,f,i,o)
    def gperm(m):
        return np.concatenate(
            [m[2 * H:3 * H], m[H:2 * H], m[0:H], m[3 * H:4 * H]], axis=0
        )

    # h stored as 2h -> h-consuming weights x0.5
    Wcomb = gperm(np.concatenate([Wctx, 0.5 * Whh], axis=1))  # [4H, E+H]
    Wemb_p = gperm(Wemb)
    bcomb_v = gperm((bih + bhh).reshape(-1, 1)).reshape(-1)
    # pre-halve f,i,o rows: sigma(x) = 0.5*tanh(x/2)+0.5 then needs only
    # tanh at scale=1 for ALL gate chunks (single ACT call)
    Wcomb[H:] *= 0.5
    Wemb_p[H:] *= 0.5
    bcomb_v[H:] *= 0.5

    def bft(m):  # [K, M] fp32 -> [128, K//128, M] bf16 tiles
        return np.ascontiguousarray(_to_tiles(m)).astype(bf16)

    WfT_h = bft(Wf.T)
    WhT_h = bft(0.5 * Wh.T)
    WcombT_h = bft(Wcomb.T)
    WembT_h = bft(Wemb_p.T)
    ihWT_h = bft(2.0 * ihW.T / P)
    icWT_h = bft(2.0 * icW.T / P)

    def pexp(vec, reps):  # [D] -> [128, D//128, reps] f32
        return np.repeat(
            vec.reshape(-1, 128).T[:, :, None], reps, axis=2
        ).astype(f32)

    ihb_h = pexp(2.0 * ihb, BL)
    icb_h = pexp(2.0 * icb, BL)
    attnb_h = pexp(attnb, BL)
    bcomb_h = pexp(bcomb_v, 1)
    eye4_h = np.eye(4, dtype=bf16)

    vmask = np.zeros((128, KH, BL, BL), np.float32)
    vt = vw.reshape(KH, 128).T                            # [128, KH]
    for b in range(BL):
        vmask[:, :, b, b] = vt
    vmask_h = vmask.astype(bf16)

    in_maps = []
    for k in range(n_cores):
        b0 = k * BL
        fk = feats[b0:b0 + BL]                            # [BL, P, E]
        featT = (
            fk.transpose(2, 0, 1)
            .reshape(KH, 128, BL, P)
            .transpose(1, 0, 2, 3)
        )
        featpad = np.zeros((BL, 2, 128, E), f32)
        featpad[:, 0] = fk[:, 0:128]
        featpad[:, 1, 0:PC1] = fk[:, 128:P]
        feat_h = featpad.transpose(2, 0, 1, 3)            # [128, BL, 2, E]
        embk = embW[caps[b0:b0 + BL]]                     # [BL, T, E]
        embT = (
            embk.transpose(2, 0, 1)
            .reshape(KH, 128, BL * T)
            .transpose(1, 0, 2)
        )
        linWT_k = _to_tiles(0.5 * linW[k * VS:(k + 1) * VS].T)
        linb_k = np.repeat(
            linb[k * VS:(k + 1) * VS][None, :], 128, axis=0
        ).astype(f32)
        in_maps.append({
            "featT": np.ascontiguousarray(featT).astype(bf16),
            "feat": np.ascontiguousarray(feat_h).astype(bf16),
            "embT": np.ascontiguousarray(embT).astype(bf16),
            "linWT": np.ascontiguousarray(linWT_k).astype(bf16),
            "linb": linb_k,
            "WfT": WfT_h, "WhT": WhT_h, "WcombT": WcombT_h, "WembT": WembT_h,
            "ihWT": ihWT_h, "icWT": icWT_h,
            "ihb": ihb_h, "icb": icb_h, "attnb": attnb_h, "bcomb": bcomb_h,
            "vmaskT": vmask_h, "eye4": eye4_h,
        })
    return in_maps


def unshard(results, n_cores):
    # each core's "out": [n_cores*BL*T, VS] rows ordered (rank, b_local, t)
    shards = [
        np.asarray(results[k]["out"]).astype(np.float32)
        .reshape(n_cores, 4, BL, T // 4, VS)
        .transpose(0, 2, 1, 3, 4)
        .reshape(n_cores * BL, T, VS)
        for k in range(n_cores)
    ]
    return np.concatenate(shards, axis=-1).reshape(B, T, V)


_NC_CACHE = {}


def kernel(**inputs):
    n_cores = N_CORES
    if n_cores not in _NC_CACHE:
        _NC_CACHE[n_cores] = build_nc(n_cores)
    nc = _NC_CACHE[n_cores]
    in_maps = make_in_maps(inputs, n_cores)
    res = run_bass_kernel_spmd(nc, in_maps, list(range(n_cores)))
    return unshard(res.results, n_cores)


if __name__ == "__main__":
    import reference
    inputs = reference.setup_inputs()
    out = kernel(**{k: np.asarray(v) for k, v in inputs.items()})
    print(out.shape, out.dtype)
